# revision 17
# baseline (speedup 1.0000x reference)
# Trainium2 Bass kernel for nn_ActionModel (GINE message passing, 2 layers + heads).
#
# Strategy (8 NeuronCores, SPMD):
#   * Shard edges by dst range: core k owns dst in [k*8192, (k+1)*8192) = 8 graphs.
#   * Host sorts edges by dst, groups them into 128-dst blocks.  Within a block,
#     edges are split into two classes by src < 32768 (dma_gather indices are int16)
#     and each class is padded to a uniform number of 128-edge chunks (CBL/CBH).
#     Every chunk is block-pure and class-pure.
#   * Per-edge x[src] / h[src] rows are fetched with the Ant dma_gather Q7 custom
#     instruction (<=1024 rows per call, round-robin over 4 SWDGE queues so all four
#     Q7 core-pairs generate descriptors in parallel).  The lo/hi class gathers read
#     from the two halves of the table.
#   * Aggregation (segment_sum) is a one-hot matmul on the TensorEngine:
#       aggT[C, 128] += msg[128e, C].T @ onehot[128e, 128d]
#     with onehot generated on the VectorEngine from per-edge local-dst ids (is_equal
#     against a constant iota row).  Padded edges carry dst_local = -1 -> zero column.
#   * Messages: msg = relu(x[src] + edge_attr @ W + b).  edge_attr arrives
#     pre-permuted/transposed with a baked-in ones row so the bias rides in the same
#     matmul.  Layer 2 injects gathered h rows into PSUM with an identity matmul
#     (the add is free on PE); layer 1 adds on the VectorEngine (C=32 is cheap).
#   * After layer-1 local aggregation each core runs the node MLP for ITS 8192 nodes
#     (channel-major), transposes to row-major bf16 and AllGathers h across the 8
#     cores so layer 2's gather reads a device-local table.
#   * Heads: graphs are contiguous 1024-node ranges, fully local to one core.  Host
#     reassembles the [64,10] and [64,1024] outputs.
import os
import sys
import numpy as np

try:
    import ml_dtypes
    _BF16 = ml_dtypes.bfloat16
except ImportError:  # pragma: no cover
    import jax.numpy as jnp
    _BF16 = jnp.bfloat16

for _p in ("/opt/trn_rl_repo",):
    if os.path.isdir(_p) and _p not in sys.path:
        sys.path.append(_p)

import concourse.bass as bass
import concourse.bacc as bacc
import concourse.mybir as mybir
import concourse.tile as tile
from concourse import bass_utils

F32 = mybir.dt.float32
BF16 = mybir.dt.bfloat16
I16 = mybir.dt.int16
AF = mybir.ActivationFunctionType
ALU = mybir.AluOpType
P = 128
XCOL = 64          # x table padded to 64 f32 cols (256B rows for dma_gather)
NQ = 4             # SWDGE queues


def default_cfg():
    return dict(N=65536, E=1048576, NF=32, EF=16, H=128, A=10, B=64,
                NCORES=8, DST_BLK=128, GB=4)


def derive_cfg(cfg, CBL, CBH):
    c = dict(cfg)
    c["CBL"], c["CBH"] = CBL, CBH
    c["CB"] = CBL + CBH               # chunks (of 128 edges) per dst-block
    c["SPLIT"] = c["N"] // 2          # src class threshold (int16 range)
    c["NC"] = c["N"] // c["NCORES"]   # nodes per core
    c["NBLK"] = c["NC"] // c["DST_BLK"]
    c["NG"] = c["NBLK"] // c["GB"]    # groups per core
    c["KG"] = c["GB"] * c["CB"]       # chunk-columns per group
    c["GPC"] = c["B"] // c["NCORES"]  # graphs per core
    c["NPER"] = c["N"] // c["B"]      # nodes per graph
    assert c["NBLK"] % c["GB"] == 0 and c["NC"] % P == 0 and c["NPER"] * c["GPC"] == c["NC"]
    return c


# ---------------------------------------------------------------- host prep
def _wrap_idx16(flat):
    """dma_gather idx layout: [128, S/16] int16, unwrapped[i] = idx[i%16, i//16],
    replicated down the 128 partitions every 16."""
    S = flat.shape[0]
    assert S % 16 == 0
    blk = flat.reshape(S // 16, 16).T        # [16, S/16]
    return np.ascontiguousarray(np.tile(blk, (8, 1))).astype(np.int16)


def prep_inputs(cfg, inputs):
    """Shard + sort + pad on host.  Returns (ccfg, in_maps)."""
    N, E = cfg["N"], cfg["E"]
    NF, EF, H = cfg["NF"], cfg["EF"], cfg["H"]
    NCORES, DST_BLK, GB = cfg["NCORES"], cfg["DST_BLK"], cfg["GB"]
    NC = N // NCORES
    NBLK = NC // DST_BLK
    SPLIT = N // 2

    x = np.asarray(inputs["x"], np.float32)
    ei = np.asarray(inputs["edge_index"])
    ea = np.asarray(inputs["edge_attr"], np.float32)
    src_all = ei[0].astype(np.int64)
    dst_all = ei[1].astype(np.int64)

    cores = []
    max_lo = max_hi = 0
    for k in range(NCORES):
        m = (dst_all >= k * NC) & (dst_all < (k + 1) * NC)
        eidx = np.nonzero(m)[0]
        dl = dst_all[eidx] - k * NC
        cls = (src_all[eidx] >= SPLIT).astype(np.int64)
        blk = dl // DST_BLK
        order = np.lexsort((cls, blk))
        eidx, dl, cls, blk = eidx[order], dl[order], cls[order], blk[order]
        cnt_lo = np.bincount(blk[cls == 0], minlength=NBLK)
        cnt_hi = np.bincount(blk[cls == 1], minlength=NBLK)
        max_lo = max(max_lo, int(cnt_lo.max()))
        max_hi = max(max_hi, int(cnt_hi.max()))
        cores.append((eidx, dl, cls, blk, cnt_lo, cnt_hi))

    CBL = max(2, -(-max_lo // P))
    CBH = max(2, -(-max_hi // P))
    ccfg = derive_cfg(cfg, CBL, CBH)
    NG, KG, CB = ccfg["NG"], ccfg["KG"], ccfg["CB"]
    LO_COLS, HI_COLS = GB * CBL, GB * CBH

    ident = np.eye(P, dtype=_BF16)
    iota_b = np.ascontiguousarray(
        np.broadcast_to(np.arange(DST_BLK, dtype=np.float32), (P, DST_BLK))).astype(_BF16)
    x_pad = np.zeros((N, XCOL), np.float32)
    x_pad[:, :NF] = x

    def bf(a):
        return np.asarray(a, np.float32).astype(_BF16)

    def b2(a, d):
        return np.asarray(a, np.float32).reshape(d, 1)

    shared = dict(
        x_pad=x_pad, ident=ident, iota_b=iota_b,
        e1w=np.concatenate([np.asarray(inputs["e1_w"], np.float32),
                            np.asarray(inputs["e1_b"], np.float32)[None]], 0).astype(_BF16),
        e2w=np.concatenate([np.asarray(inputs["e2_w"], np.float32),
                            np.asarray(inputs["e2_b"], np.float32)[None]], 0).astype(_BF16),
        c1w1=bf(inputs["c1_w1"]), c1w2=bf(inputs["c1_w2"]),
        c2w1=bf(inputs["c2_w1"]), c2w2=bf(inputs["c2_w2"]),
        aw1=bf(inputs["a_w1"]), aw2=bf(inputs["a_w2"]),
        nw1=bf(inputs["n_w1"]), nw2=bf(inputs["n_w2"]), nw3=bf(inputs["n_w3"]),
        c1b1=b2(inputs["c1_b1"], H), c1b2=b2(inputs["c1_b2"], H),
        c2b1=b2(inputs["c2_b1"], H), c2b2=b2(inputs["c2_b2"], H),
        ab1=b2(inputs["a_b1"], H), ab2=b2(inputs["a_b2"], cfg["A"]),
        nb1=b2(inputs["n_b1"], H), nb2=b2(inputs["n_b2"], H),
        nb3=b2(inputs["n_b3"], 1),
    )

    in_maps = []
    for k in range(NCORES):
        eidx, dl, cls, blk, cnt_lo, cnt_hi = cores[k]
        starts_lo = np.zeros(NBLK + 1, np.int64); np.cumsum(cnt_lo, out=starts_lo[1:])
        starts_hi = np.zeros(NBLK + 1, np.int64); np.cumsum(cnt_hi, out=starts_hi[1:])
        # within (block, class) ranks: edges are sorted by (blk, cls)
        r = np.arange(len(dl))
        base = np.where(cls == 0,
                        starts_lo[blk] + starts_hi[blk],
                        starts_lo[blk + 1] + starts_hi[blk])
        r_in = r - base
        g = blk // GB
        b = blk % GB
        ci = r_in >> 7
        p = r_in & 127
        col = np.where(cls == 0, b * CBL + ci, LO_COLS + b * CBH + ci)

        dstloc = np.full((NG, P, KG), -1.0, np.float32)  # cast to bf16 below
        eaT = np.zeros((NG, EF + 1, KG * P), np.float32)
        eaT[:, EF, :] = 1.0
        dstloc[g, p, col] = (dl % DST_BLK).astype(np.float32)
        eaT[g.reshape(-1, 1), np.arange(EF).reshape(1, -1),
            (col * P + p).reshape(-1, 1)] = ea[eidx]

        idx_lo = np.zeros((NG, LO_COLS * P), np.int64)
        idx_hi = np.zeros((NG, HI_COLS * P), np.int64)
        lo_m, hi_m = cls == 0, cls == 1
        idx_lo[g[lo_m], (col[lo_m]) * P + p[lo_m]] = src_all[eidx[lo_m]]
        idx_hi[g[hi_m], (col[hi_m] - LO_COLS) * P + p[hi_m]] = src_all[eidx[hi_m]] - SPLIT
        im = dict(shared)
        im["idx_lo"] = np.stack([_wrap_idx16(idx_lo[gg]) for gg in range(NG)])
        im["idx_hi"] = np.stack([_wrap_idx16(idx_hi[gg]) for gg in range(NG)])
        im["dstloc"] = dstloc.astype(_BF16)
        im["eaT"] = eaT.astype(_BF16)
        im["xT"] = np.ascontiguousarray(x[k * NC:(k + 1) * NC].T)
        in_maps.append(im)
    return ccfg, in_maps


# ---------------------------------------------------------------- device code
def build_device_kernel(tc, outs, ins, cfg):
    nc = tc.nc
    N, NF, EF, H, A = cfg["N"], cfg["NF"], cfg["EF"], cfg["H"], cfg["A"]
    NC, NG, GB, KG = cfg["NC"], cfg["NG"], cfg["GB"], cfg["KG"]
    CBL, CBH, CB = cfg["CBL"], cfg["CBH"], cfg["CB"]
    DST_BLK, GPC, NPER, SPLIT = cfg["DST_BLK"], cfg["GPC"], cfg["NPER"], cfg["SPLIT"]
    NTILE = NC // P
    LO_COLS, HI_COLS = GB * CBL, GB * CBH
    qctr = [0]

    from contextlib import ExitStack
    with ExitStack() as ctx:
        const = ctx.enter_context(tc.tile_pool(name="const", bufs=1))
        big = ctx.enter_context(tc.tile_pool(name="big", bufs=1))
        dram = ctx.enter_context(tc.tile_pool(name="dram", bufs=1, space="DRAM"))

        def load_const(name, shape, dtype):
            t = const.tile(list(shape), dtype, tag=name)
            nc.sync.dma_start(out=t[:], in_=ins[name][:])
            return t

        ident = load_const("ident", [P, P], BF16)
        iota_b = load_const("iota_b", [P, DST_BLK], BF16)
        e1w = load_const("e1w", [EF + 1, NF], BF16)
        e2w = load_const("e2w", [EF + 1, H], BF16)
        c1w1 = load_const("c1w1", [NF, H], BF16)
        c1w2 = load_const("c1w2", [H, H], BF16)
        c2w1 = load_const("c2w1", [H, H], BF16)
        c2w2 = load_const("c2w2", [H, H], BF16)
        aw1 = load_const("aw1", [H, H], BF16)
        aw2 = load_const("aw2", [H, A], BF16)
        nw1 = load_const("nw1", [H, H], BF16)
        nw2 = load_const("nw2", [H, H], BF16)
        nw3 = load_const("nw3", [H, 1], BF16)
        c1b1 = load_const("c1b1", [H, 1], F32)
        c1b2 = load_const("c1b2", [H, 1], F32)
        c2b1 = load_const("c2b1", [H, 1], F32)
        c2b2 = load_const("c2b2", [H, 1], F32)
        ab1 = load_const("ab1", [H, 1], F32)
        ab2 = load_const("ab2", [A, 1], F32)
        nb1 = load_const("nb1", [H, 1], F32)
        nb2 = load_const("nb2", [H, 1], F32)
        nb3 = load_const("nb3", [1, 1], F32)
        xT = load_const("xT", [NF, NC], F32)

        hpre = big.tile([NF, NC], BF16, tag="hpre")    # x + agg1 (channel-major)
        hT = big.tile([H, NC], BF16, tag="hT")         # relu(mlp1) local, channel-major
        h2pre = big.tile([H, NC], BF16, tag="h2pre")
        h2T = big.tile([H, NC], BF16, tag="h2T")
        nh = big.tile([H, NC], BF16, tag="hpre")  # reuses dead hpre space

        h_shard = dram.tile([NC, H], BF16, tag="h_shard")
        h_full = dram.tile([N, H], BF16, tag="h_full", addr_space="Shared")

        def gathers(gt3, it, table, ncols, Ce):
            """Issue <=1024-row dma_gather calls covering ncols chunk-columns."""
            off = 0
            while off < ncols:
                n = min(8, ncols - off)
                ni = n * P
                nc.gpsimd.dma_gather(
                    out_ap=gt3[:, off:off + n, :],
                    in_ap=table,
                    idxs_ap=it[:, off * 8:off * 8 + n * 8],
                    num_idxs=ni, num_idxs_reg=ni, elem_size=Ce,
                    queue_num=qctr[0] % NQ)
                qctr[0] += 1
                off += n

        def edge_phase(layer, block_done):
            C = NF if layer == 1 else H
            Ce = XCOL if layer == 1 else H          # gathered row width
            DT = F32 if layer == 1 else BF16
            ew = e1w if layer == 1 else e2w
            tbl = ins["x_pad"] if layer == 1 else h_full
            with ExitStack() as ectx:
                io = ectx.enter_context(tc.tile_pool(name=f"io{layer}", bufs=2))
                gtp = ectx.enter_context(tc.tile_pool(name=f"gtp{layer}", bufs=3))
                sb = ectx.enter_context(tc.tile_pool(name=f"sb{layer}", bufs=3))
                msgp = ectx.enter_context(
                    tc.tile_pool(name=f"msgp{layer}", bufs=3, space="PSUM"))
                aggp = ectx.enter_context(
                    tc.tile_pool(name=f"aggp{layer}", bufs=2, space="PSUM"))
                msb = ectx.enter_context(tc.tile_pool(name=f"m{layer}sb", bufs=3))
                mps = ectx.enter_context(
                    tc.tile_pool(name=f"m{layer}ps", bufs=3, space="PSUM"))
                for g in range(NG):
                    it_lo = io.tile([P, LO_COLS * 8], I16, tag="itlo")
                    nc.sync.dma_start(out=it_lo[:], in_=ins["idx_lo"][g])
                    it_hi = io.tile([P, HI_COLS * 8], I16, tag="ithi")
                    nc.sync.dma_start(out=it_hi[:], in_=ins["idx_hi"][g])
                    dl_t = io.tile([P, KG], BF16, tag="dl")
                    nc.sync.dma_start(out=dl_t[:], in_=ins["dstloc"][g])
                    ea_t = io.tile([EF + 1, KG * P], BF16, tag="ea")
                    nc.sync.dma_start(out=ea_t[:], in_=ins["eaT"][g])
                    gt = gtp.tile([P, KG * Ce], DT, tag="gt")
                    gt3 = gt[:].rearrange("p (n c) -> p n c", c=Ce)
                    gathers(gt3[:, 0:LO_COLS, :], it_lo, tbl[0:SPLIT, :], LO_COLS, Ce)
                    gathers(gt3[:, LO_COLS:KG, :], it_hi, tbl[SPLIT:N, :], HI_COLS, Ce)
                    for b in range(GB):
                        agg_t = aggp.tile([C, DST_BLK], F32, tag="agg")
                        # quads: runs of <=4 chunk-columns, class-pure & contiguous
                        quads = []
                        for c0 in range(0, CBL, 4):
                            quads.append((b * CBL + c0, min(4, CBL - c0)))
                        for c0 in range(0, CBH, 4):
                            quads.append((LO_COLS + b * CBH + c0, min(4, CBH - c0)))
                        ci = 0
                        for (col0, q) in quads:
                            mm = msgp.tile([P, 4 * C], F32, tag="mm")
                            msg_t = sb.tile([P, 4 * C], BF16, tag="msg")
                            if layer == 2:
                                nc.tensor.matmul(mm[:, 0:q * C], lhsT=ident[:],
                                                 rhs=gt[:, col0 * Ce:(col0 + q) * Ce],
                                                 start=True, stop=False,
                                                 skip_group_check=True)
                                for j in range(q):
                                    nc.tensor.matmul(
                                        mm[:, j * C:(j + 1) * C],
                                        lhsT=ea_t[:, (col0 + j) * P:(col0 + j + 1) * P],
                                        rhs=ew[:], start=False, stop=True,
                                        skip_group_check=True)
                                nc.scalar.activation(msg_t[:, 0:q * C],
                                                     mm[:, 0:q * C], AF.Relu)
                            else:
                                for j in range(q):
                                    nc.tensor.matmul(
                                        mm[:, j * C:(j + 1) * C],
                                        lhsT=ea_t[:, (col0 + j) * P:(col0 + j + 1) * P],
                                        rhs=ew[:], start=True, stop=True,
                                        skip_group_check=True)
                                tmp = sb.tile([P, 4 * C], F32, tag="tmp")
                                nc.vector.tensor_tensor(
                                    out=tmp[:, 0:q * C],
                                    in0=gt3[:, col0:col0 + q, 0:NF],
                                    in1=mm[:, 0:q * C], op=ALU.add)
                                nc.scalar.activation(msg_t[:, 0:q * C],
                                                     tmp[:, 0:q * C], AF.Relu)
                            for j in range(q):
                                col = col0 + j
                                oh_t = sb.tile([P, DST_BLK], BF16, tag="oh")
                                nc.vector.tensor_tensor(
                                    out=oh_t[:], in0=iota_b[:],
                                    in1=dl_t[:, col:col + 1].to_broadcast([P, DST_BLK]),
                                    op=ALU.is_equal)
                                nc.tensor.matmul(agg_t[:],
                                                 lhsT=msg_t[:, j * C:(j + 1) * C],
                                                 rhs=oh_t[:],
                                                 start=(ci == 0), stop=(ci == CB - 1))
                                ci += 1
                        colb = (g * GB + b) * DST_BLK
                        dst_sl = slice(colb, colb + DST_BLK)
                        if layer == 1:
                            nc.vector.tensor_tensor(
                                out=hpre[:, dst_sl], in0=xT[:, dst_sl],
                                in1=agg_t[:], op=ALU.add)
                        else:
                            nc.vector.tensor_tensor(
                                out=h2pre[:, dst_sl], in0=hT[:, dst_sl],
                                in1=agg_t[:], op=ALU.add)
                        block_done(g * GB + b, msb, mps)

        def leaky(pool, src_ps, bias, out_ap, width, parts=P):
            t = pool.tile([parts, width], F32, tag="lk")
            nc.scalar.activation(t[:], src_ps[:], AF.Identity, bias=bias[:, 0:1])
            nc.vector.scalar_tensor_tensor(
                out=out_ap, in0=t[:], scalar=0.01, in1=t[:],
                op0=ALU.mult, op1=ALU.max)

        # ---------------- layer 1 ----------------
        def mlp1_block(t, msb, mps):
            s = slice(t * P, (t + 1) * P)
            ps1 = mps.tile([H, P], F32, tag="ps")
            nc.tensor.matmul(ps1[:], lhsT=c1w1[:], rhs=hpre[:, s],
                             start=True, stop=True)
            l1 = msb.tile([H, P], BF16, tag="l1")
            leaky(msb, ps1, c1b1, l1[:], P)
            ps2 = mps.tile([H, P], F32, tag="ps")
            nc.tensor.matmul(ps2[:], lhsT=c1w2[:], rhs=l1[:],
                             start=True, stop=True)
            nc.scalar.activation(hT[:, s], ps2[:], AF.Relu, bias=c1b2[:, 0:1])
            tr = mps.tile([P, H], BF16, tag="ps")
            nc.tensor.transpose(tr[:], hT[:, s], ident[:])
            hrow = msb.tile([P, H], BF16, tag="hrow")
            nc.vector.tensor_copy(out=hrow[:], in_=tr[:])
            nc.sync.dma_start(out=h_shard[s, :], in_=hrow[:])

        edge_phase(1, mlp1_block)

        nc.gpsimd.collective_compute(
            "AllGather", ALU.bypass,
            replica_groups=[list(range(cfg["NCORES"]))],
            ins=[h_shard[:, :]], outs=[h_full[:, :]])

        # ---------------- layer 2 ----------------
        NS_STEP = min(512, NC)
        NS_BLKS = NS_STEP // P

        def mlp2_block(t, msb, mps):
            s = slice(t * P, (t + 1) * P)
            ps1 = mps.tile([H, P], F32, tag="ps")
            nc.tensor.matmul(ps1[:], lhsT=c2w1[:], rhs=h2pre[:, s],
                             start=True, stop=True)
            l1 = msb.tile([H, P], BF16, tag="l1")
            leaky(msb, ps1, c2b1, l1[:], P)
            ps2 = mps.tile([H, P], F32, tag="ps")
            nc.tensor.matmul(ps2[:], lhsT=c2w2[:], rhs=l1[:],
                             start=True, stop=True)
            leaky(msb, ps2, c2b2, h2T[:, s], P)
            ps3 = mps.tile([H, P], F32, tag="ps")
            nc.tensor.matmul(ps3[:], lhsT=nw1[:], rhs=h2T[:, s],
                             start=True, stop=True)
            n1 = msb.tile([H, P], BF16, tag="n1")
            leaky(msb, ps3, nb1, n1[:], P)
            ps4 = mps.tile([H, P], F32, tag="ps")
            nc.tensor.matmul(ps4[:], lhsT=nw2[:], rhs=n1[:],
                             start=True, stop=True)
            leaky(msb, ps4, nb2, nh[:, s], P)
            if (t + 1) % NS_BLKS == 0:
                s0 = (t + 1) * P - NS_STEP
                psn = mps.tile([1, NS_STEP], F32, tag="ps")
                nc.tensor.matmul(psn[:], lhsT=nw3[:], rhs=nh[:, s0:s0 + NS_STEP],
                                 start=True, stop=True)
                nst = msb.tile([1, NS_STEP], F32, tag="nst")
                nc.scalar.activation(nst[:], psn[:], AF.Sigmoid, bias=nb3[:, 0:1])
                nc.sync.dma_start(out=outs["out_ns"][0:1, s0:s0 + NS_STEP], in_=nst[:])

        edge_phase(2, mlp2_block)
        with ExitStack() as mctx:
            msb = mctx.enter_context(tc.tile_pool(name="ahsb", bufs=2))
            mps = mctx.enter_context(tc.tile_pool(name="ahps", bufs=2, space="PSUM"))
            # ---------------- action head ----------------
            pooled = msb.tile([H, GPC], F32, tag="pooled")
            for gi in range(GPC):
                nc.vector.tensor_reduce(
                    out=pooled[:, gi:gi + 1],
                    in_=h2T[:, gi * NPER:(gi + 1) * NPER],
                    axis=mybir.AxisListType.X, op=ALU.add)
            pooled_bf = msb.tile([H, GPC], BF16, tag="pooled_bf")
            nc.vector.tensor_scalar(out=pooled_bf[:], in0=pooled[:],
                                    scalar1=1.0 / NPER, scalar2=None, op0=ALU.mult)
            psa = mps.tile([H, GPC], F32, tag="ps")
            nc.tensor.matmul(psa[:], lhsT=aw1[:], rhs=pooled_bf[:],
                             start=True, stop=True)
            a1 = msb.tile([H, GPC], BF16, tag="a1")
            leaky(msb, psa, ab1, a1[:], GPC)
            psa2 = mps.tile([A, GPC], F32, tag="ps")
            nc.tensor.matmul(psa2[:], lhsT=aw2[:], rhs=a1[:],
                             start=True, stop=True)
            a2 = msb.tile([A, GPC], BF16, tag="a2")
            leaky(msb, psa2, ab2, a2[:], GPC, parts=A)
            trp = mps.tile([GPC, A], BF16, tag="ps")
            nc.tensor.transpose(trp[:], a2[:], ident[:A, :A])
            sm = msb.tile([GPC, A], F32, tag="sm")
            nc.vector.tensor_copy(out=sm[:], in_=trp[:])
            negmax = msb.tile([GPC, 1], F32, tag="negmax")
            nc.vector.tensor_reduce(out=negmax[:], in_=sm[:],
                                    axis=mybir.AxisListType.X, op=ALU.max,
                                    negate=True)
            ex = msb.tile([GPC, A], F32, tag="ex")
            nc.scalar.activation(ex[:], sm[:], AF.Exp, bias=negmax[:, 0:1])
            ssum = msb.tile([GPC, 1], F32, tag="ssum")
            nc.vector.tensor_reduce(out=ssum[:], in_=ex[:],
                                    axis=mybir.AxisListType.X, op=ALU.add)
            rinv = msb.tile([GPC, 1], F32, tag="rinv")
            nc.vector.reciprocal(rinv[:], ssum[:])
            act = msb.tile([GPC, A], F32, tag="act")
            nc.vector.tensor_scalar(out=act[:], in0=ex[:],
                                    scalar1=rinv[:, 0:1], scalar2=None,
                                    op0=ALU.mult)
            nc.sync.dma_start(out=outs["out_act"][:, :], in_=act[:])


# ---------------------------------------------------------------- driver
def build_program(ccfg):
    nc = bacc.Bacc("TRN2", target_bir_lowering=False, debug=False,
                   num_devices=ccfg["NCORES"], num_swdge_queues=NQ)
    N, NF, EF, H, A = ccfg["N"], ccfg["NF"], ccfg["EF"], ccfg["H"], ccfg["A"]
    NC, NG, KG, GPC = ccfg["NC"], ccfg["NG"], ccfg["KG"], ccfg["GPC"]
    DST_BLK, GB = ccfg["DST_BLK"], ccfg["GB"]
    LO_COLS, HI_COLS = GB * ccfg["CBL"], GB * ccfg["CBH"]

    def din(name, shape, dt):
        return nc.dram_tensor(name, list(shape), dt, kind="ExternalInput").ap()

    ins = dict(
        x_pad=din("x_pad", [N, XCOL], F32),
        xT=din("xT", [NF, NC], F32),
        idx_lo=din("idx_lo", [NG, P, LO_COLS * 8], I16),
        idx_hi=din("idx_hi", [NG, P, HI_COLS * 8], I16),
        dstloc=din("dstloc", [NG, P, KG], BF16),
        eaT=din("eaT", [NG, EF + 1, KG * P], BF16),
        ident=din("ident", [P, P], BF16),
        iota_b=din("iota_b", [P, DST_BLK], BF16),
        e1w=din("e1w", [EF + 1, NF], BF16),
        e2w=din("e2w", [EF + 1, H], BF16),
        c1w1=din("c1w1", [NF, H], BF16),
        c1w2=din("c1w2", [H, H], BF16),
        c2w1=din("c2w1", [H, H], BF16),
        c2w2=din("c2w2", [H, H], BF16),
        aw1=din("aw1", [H, H], BF16),
        aw2=din("aw2", [H, A], BF16),
        nw1=din("nw1", [H, H], BF16),
        nw2=din("nw2", [H, H], BF16),
        nw3=din("nw3", [H, 1], BF16),
        c1b1=din("c1b1", [H, 1], F32),
        c1b2=din("c1b2", [H, 1], F32),
        c2b1=din("c2b1", [H, 1], F32),
        c2b2=din("c2b2", [H, 1], F32),
        ab1=din("ab1", [H, 1], F32),
        ab2=din("ab2", [A, 1], F32),
        nb1=din("nb1", [H, 1], F32),
        nb2=din("nb2", [H, 1], F32),
        nb3=din("nb3", [1, 1], F32),
    )
    outs = dict(
        out_act=nc.dram_tensor("out_act", [GPC, A], F32, kind="ExternalOutput").ap(),
        out_ns=nc.dram_tensor("out_ns", [1, NC], F32, kind="ExternalOutput").ap(),
    )
    with tile.TileContext(nc) as tc:
        build_device_kernel(tc, outs, ins, ccfg)
    nc.compile()
    return nc


_PROGRAM_CACHE = {}


def assemble_outputs(cfg, results):
    B, A, NPER = cfg["B"], cfg["A"], cfg["NPER"]
    NCORES = cfg["NCORES"]
    action = np.concatenate([results[k]["out_act"] for k in range(NCORES)], 0)
    ns_all = np.concatenate(
        [results[k]["out_ns"].reshape(-1) for k in range(NCORES)], 0)
    node_scores = np.ascontiguousarray(ns_all.reshape(NPER, B).T)
    return action.astype(np.float32), node_scores.astype(np.float32)


def kernel(**inputs):
    cfg = default_cfg()
    ccfg, in_maps = prep_inputs(cfg, inputs)
    key = (ccfg["CBL"], ccfg["CBH"])
    if key not in _PROGRAM_CACHE:
        _PROGRAM_CACHE[key] = build_program(ccfg)
    nc = _PROGRAM_CACHE[key]
    res = bass_utils.run_bass_kernel_spmd(
        nc, in_maps, core_ids=list(range(ccfg["NCORES"])))
    return assemble_outputs(ccfg, res.results)


# revision 18
# speedup vs baseline: 1.1150x; 1.1150x over previous
# Trainium2 Bass kernel for nn_ActionModel (GINE message passing, 2 layers + heads).
#
# Strategy (8 NeuronCores, SPMD):
#   * Shard edges by dst range: core k owns dst in [k*8192, (k+1)*8192) = 8 graphs.
#   * Host sorts edges by dst, groups them into 128-dst blocks.  Within a block,
#     edges are split into two classes by src < 32768 (dma_gather indices are int16)
#     and each class is padded to a uniform number of 128-edge chunks (CBL/CBH).
#     Every chunk is block-pure and class-pure.
#   * Per-edge x[src] / h[src] rows are fetched with the Ant dma_gather Q7 custom
#     instruction (<=1024 rows per call, round-robin over 4 SWDGE queues so all four
#     Q7 core-pairs generate descriptors in parallel).  The lo/hi class gathers read
#     from the two halves of the table.
#   * Aggregation (segment_sum) is a one-hot matmul on the TensorEngine:
#       aggT[C, 128] += msg[128e, C].T @ onehot[128e, 128d]
#     with onehot generated on the VectorEngine from per-edge local-dst ids (is_equal
#     against a constant iota row).  Padded edges carry dst_local = -1 -> zero column.
#   * Messages: msg = relu(x[src] + edge_attr @ W + b).  edge_attr arrives
#     pre-permuted/transposed with a baked-in ones row so the bias rides in the same
#     matmul.  Layer 2 injects gathered h rows into PSUM with an identity matmul
#     (the add is free on PE); layer 1 adds on the VectorEngine (C=32 is cheap).
#   * After layer-1 local aggregation each core runs the node MLP for ITS 8192 nodes
#     (channel-major), transposes to row-major bf16 and AllGathers h across the 8
#     cores so layer 2's gather reads a device-local table.
#   * Heads: graphs are contiguous 1024-node ranges, fully local to one core.  Host
#     reassembles the [64,10] and [64,1024] outputs.
import os
import sys
import numpy as np

try:
    import ml_dtypes
    _BF16 = ml_dtypes.bfloat16
except ImportError:  # pragma: no cover
    import jax.numpy as jnp
    _BF16 = jnp.bfloat16

for _p in ("/opt/trn_rl_repo",):
    if os.path.isdir(_p) and _p not in sys.path:
        sys.path.append(_p)

import concourse.bass as bass
import concourse.bacc as bacc
import concourse.mybir as mybir
import concourse.tile as tile
from concourse import bass_utils

F32 = mybir.dt.float32
BF16 = mybir.dt.bfloat16
I16 = mybir.dt.int16
AF = mybir.ActivationFunctionType
ALU = mybir.AluOpType
P = 128
XCOL = 64          # x table padded to 64 f32 cols (256B rows for dma_gather)
NQ = 4             # SWDGE queues


def default_cfg():
    return dict(N=65536, E=1048576, NF=32, EF=16, H=128, A=10, B=64,
                NCORES=8, DST_BLK=128, GB=4)


def derive_cfg(cfg, CBL, CBH):
    c = dict(cfg)
    c["CBL"], c["CBH"] = CBL, CBH
    c["CB"] = CBL + CBH               # chunks (of 128 edges) per dst-block
    c["SPLIT"] = c["N"] // 2          # src class threshold (int16 range)
    c["NC"] = c["N"] // c["NCORES"]   # nodes per core
    c["NBLK"] = c["NC"] // c["DST_BLK"]
    c["NG"] = c["NBLK"] // c["GB"]    # groups per core
    c["KG"] = c["GB"] * c["CB"]       # chunk-columns per group
    c["GPC"] = c["B"] // c["NCORES"]  # graphs per core
    c["NPER"] = c["N"] // c["B"]      # nodes per graph
    assert c["NBLK"] % c["GB"] == 0 and c["NC"] % P == 0 and c["NPER"] * c["GPC"] == c["NC"]
    return c


# ---------------------------------------------------------------- host prep
def _wrap_idx16(flat):
    """dma_gather idx layout: [128, S/16] int16, unwrapped[i] = idx[i%16, i//16],
    replicated down the 128 partitions every 16."""
    S = flat.shape[0]
    assert S % 16 == 0
    blk = flat.reshape(S // 16, 16).T        # [16, S/16]
    return np.ascontiguousarray(np.tile(blk, (8, 1))).astype(np.int16)


def prep_inputs(cfg, inputs):
    """Shard + sort + pad on host.  Returns (ccfg, in_maps)."""
    N, E = cfg["N"], cfg["E"]
    NF, EF, H = cfg["NF"], cfg["EF"], cfg["H"]
    NCORES, DST_BLK, GB = cfg["NCORES"], cfg["DST_BLK"], cfg["GB"]
    NC = N // NCORES
    NBLK = NC // DST_BLK
    SPLIT = N // 2

    x = np.asarray(inputs["x"], np.float32)
    ei = np.asarray(inputs["edge_index"])
    ea = np.asarray(inputs["edge_attr"], np.float32)
    src_all = ei[0].astype(np.int64)
    dst_all = ei[1].astype(np.int64)

    cores = []
    max_lo = max_hi = 0
    for k in range(NCORES):
        m = (dst_all >= k * NC) & (dst_all < (k + 1) * NC)
        eidx = np.nonzero(m)[0]
        dl = dst_all[eidx] - k * NC
        cls = (src_all[eidx] >= SPLIT).astype(np.int64)
        blk = dl // DST_BLK
        order = np.lexsort((cls, blk))
        eidx, dl, cls, blk = eidx[order], dl[order], cls[order], blk[order]
        cnt_lo = np.bincount(blk[cls == 0], minlength=NBLK)
        cnt_hi = np.bincount(blk[cls == 1], minlength=NBLK)
        max_lo = max(max_lo, int(cnt_lo.max()))
        max_hi = max(max_hi, int(cnt_hi.max()))
        cores.append((eidx, dl, cls, blk, cnt_lo, cnt_hi))

    CBL = max(2, -(-max_lo // P))
    CBH = max(2, -(-max_hi // P))
    ccfg = derive_cfg(cfg, CBL, CBH)
    NG, KG, CB = ccfg["NG"], ccfg["KG"], ccfg["CB"]
    LO_COLS, HI_COLS = GB * CBL, GB * CBH

    ident = np.eye(P, dtype=_BF16)
    iota_b = np.ascontiguousarray(
        np.broadcast_to(np.arange(DST_BLK, dtype=np.float32), (P, DST_BLK))).astype(_BF16)
    x_pad = np.zeros((N, XCOL), np.float32)
    x_pad[:, :NF] = x

    def bf(a):
        return np.asarray(a, np.float32).astype(_BF16)

    def b2(a, d):
        return np.asarray(a, np.float32).reshape(d, 1)

    shared = dict(
        x_pad=x_pad, ident=ident, iota_b=iota_b,
        e1w=np.concatenate([np.asarray(inputs["e1_w"], np.float32),
                            np.asarray(inputs["e1_b"], np.float32)[None]], 0).astype(_BF16),
        e2w=np.concatenate([np.asarray(inputs["e2_w"], np.float32),
                            np.asarray(inputs["e2_b"], np.float32)[None]], 0).astype(_BF16),
        c1w1=bf(inputs["c1_w1"]), c1w2=bf(inputs["c1_w2"]),
        c2w1=bf(inputs["c2_w1"]), c2w2=bf(inputs["c2_w2"]),
        aw1=bf(inputs["a_w1"]), aw2=bf(inputs["a_w2"]),
        nw1=bf(inputs["n_w1"]), nw2=bf(inputs["n_w2"]), nw3=bf(inputs["n_w3"]),
        c1b1=b2(inputs["c1_b1"], H), c1b2=b2(inputs["c1_b2"], H),
        c2b1=b2(inputs["c2_b1"], H), c2b2=b2(inputs["c2_b2"], H),
        ab1=b2(inputs["a_b1"], H), ab2=b2(inputs["a_b2"], cfg["A"]),
        nb1=b2(inputs["n_b1"], H), nb2=b2(inputs["n_b2"], H),
        nb3=b2(inputs["n_b3"], 1),
    )

    in_maps = []
    for k in range(NCORES):
        eidx, dl, cls, blk, cnt_lo, cnt_hi = cores[k]
        starts_lo = np.zeros(NBLK + 1, np.int64); np.cumsum(cnt_lo, out=starts_lo[1:])
        starts_hi = np.zeros(NBLK + 1, np.int64); np.cumsum(cnt_hi, out=starts_hi[1:])
        # within (block, class) ranks: edges are sorted by (blk, cls)
        r = np.arange(len(dl))
        base = np.where(cls == 0,
                        starts_lo[blk] + starts_hi[blk],
                        starts_lo[blk + 1] + starts_hi[blk])
        r_in = r - base
        g = blk // GB
        b = blk % GB
        ci = r_in >> 7
        p = r_in & 127
        col = np.where(cls == 0, b * CBL + ci, LO_COLS + b * CBH + ci)

        dstloc = np.full((NG, P, KG), -1.0, np.float32)  # cast to bf16 below
        eaT = np.zeros((NG, EF + 1, KG * P), np.float32)
        eaT[:, EF, :] = 1.0
        dstloc[g, p, col] = (dl % DST_BLK).astype(np.float32)
        eaT[g.reshape(-1, 1), np.arange(EF).reshape(1, -1),
            (col * P + p).reshape(-1, 1)] = ea[eidx]

        idx_lo = np.zeros((NG, LO_COLS * P), np.int64)
        idx_hi = np.zeros((NG, HI_COLS * P), np.int64)
        lo_m, hi_m = cls == 0, cls == 1
        idx_lo[g[lo_m], (col[lo_m]) * P + p[lo_m]] = src_all[eidx[lo_m]]
        idx_hi[g[hi_m], (col[hi_m] - LO_COLS) * P + p[hi_m]] = src_all[eidx[hi_m]] - SPLIT
        im = dict(shared)
        im["idx_lo"] = np.stack([_wrap_idx16(idx_lo[gg]) for gg in range(NG)])
        im["idx_hi"] = np.stack([_wrap_idx16(idx_hi[gg]) for gg in range(NG)])
        im["dstloc"] = dstloc.astype(_BF16)
        im["eaT"] = eaT.astype(_BF16)
        im["xT"] = np.ascontiguousarray(x[k * NC:(k + 1) * NC].T)
        in_maps.append(im)
    return ccfg, in_maps


# ---------------------------------------------------------------- device code
def build_device_kernel(tc, outs, ins, cfg):
    nc = tc.nc
    N, NF, EF, H, A = cfg["N"], cfg["NF"], cfg["EF"], cfg["H"], cfg["A"]
    NC, NG, GB, KG = cfg["NC"], cfg["NG"], cfg["GB"], cfg["KG"]
    CBL, CBH, CB = cfg["CBL"], cfg["CBH"], cfg["CB"]
    DST_BLK, GPC, NPER, SPLIT = cfg["DST_BLK"], cfg["GPC"], cfg["NPER"], cfg["SPLIT"]
    NTILE = NC // P
    LO_COLS, HI_COLS = GB * CBL, GB * CBH
    qctr = [0]

    from contextlib import ExitStack
    with ExitStack() as ctx:
        const = ctx.enter_context(tc.tile_pool(name="const", bufs=1))
        big = ctx.enter_context(tc.tile_pool(name="big", bufs=1))
        dram = ctx.enter_context(tc.tile_pool(name="dram", bufs=1, space="DRAM"))

        def load_const(name, shape, dtype):
            t = const.tile(list(shape), dtype, tag=name)
            nc.sync.dma_start(out=t[:], in_=ins[name][:])
            return t

        ident = load_const("ident", [P, P], BF16)
        iota_b = load_const("iota_b", [P, DST_BLK], BF16)
        e1w = load_const("e1w", [EF + 1, NF], BF16)
        e2w = load_const("e2w", [EF + 1, H], BF16)
        c1w1 = load_const("c1w1", [NF, H], BF16)
        c1w2 = load_const("c1w2", [H, H], BF16)
        c2w1 = load_const("c2w1", [H, H], BF16)
        c2w2 = load_const("c2w2", [H, H], BF16)
        aw1 = load_const("aw1", [H, H], BF16)
        aw2 = load_const("aw2", [H, A], BF16)
        nw1 = load_const("nw1", [H, H], BF16)
        nw2 = load_const("nw2", [H, H], BF16)
        nw3 = load_const("nw3", [H, 1], BF16)
        c1b1 = load_const("c1b1", [H, 1], F32)
        c1b2 = load_const("c1b2", [H, 1], F32)
        c2b1 = load_const("c2b1", [H, 1], F32)
        c2b2 = load_const("c2b2", [H, 1], F32)
        ab1 = load_const("ab1", [H, 1], F32)
        ab2 = load_const("ab2", [A, 1], F32)
        nb1 = load_const("nb1", [H, 1], F32)
        nb2 = load_const("nb2", [H, 1], F32)
        nb3 = load_const("nb3", [1, 1], F32)
        xT = load_const("xT", [NF, NC], F32)

        hpre = big.tile([NF, NC], BF16, tag="hpre")    # x + agg1 (channel-major)
        hT = big.tile([H, NC], BF16, tag="hT")         # relu(mlp1) local, channel-major
        h2pre = big.tile([H, NC], BF16, tag="h2pre")
        h2T = big.tile([H, NC], BF16, tag="h2T")
        nh = big.tile([H, NC], BF16, tag="hpre")  # reuses dead hpre space

        h_shard = dram.tile([NC, H], BF16, tag="h_shard")
        h_full = dram.tile([N, H], BF16, tag="h_full", addr_space="Shared")

        def gathers(gt3, it, table, ncols, Ce):
            """Issue <=1024-row dma_gather calls covering ncols chunk-columns."""
            off = 0
            while off < ncols:
                n = min(8, ncols - off)
                ni = n * P
                nc.gpsimd.dma_gather(
                    out_ap=gt3[:, off:off + n, :],
                    in_ap=table,
                    idxs_ap=it[:, off * 8:off * 8 + n * 8],
                    num_idxs=ni, num_idxs_reg=ni, elem_size=Ce,
                    queue_num=qctr[0] % NQ)
                qctr[0] += 1
                off += n

        def edge_phase(layer, block_done):
            C = NF if layer == 1 else H
            Ce = XCOL if layer == 1 else H          # gathered row width
            DT = F32 if layer == 1 else BF16
            ew = e1w if layer == 1 else e2w
            tbl = ins["x_pad"] if layer == 1 else h_full
            with ExitStack() as ectx:
                io = ectx.enter_context(tc.tile_pool(name=f"io{layer}", bufs=2))
                gtp = ectx.enter_context(tc.tile_pool(name=f"gtp{layer}", bufs=2))
                sb = ectx.enter_context(tc.tile_pool(name=f"sb{layer}", bufs=3))
                msgp = ectx.enter_context(
                    tc.tile_pool(name=f"msgp{layer}", bufs=3, space="PSUM"))
                aggp = ectx.enter_context(
                    tc.tile_pool(name=f"aggp{layer}", bufs=2, space="PSUM"))
                msb = ectx.enter_context(tc.tile_pool(name=f"m{layer}sb", bufs=3))
                mps = ectx.enter_context(
                    tc.tile_pool(name=f"m{layer}ps", bufs=3, space="PSUM"))
                for g in range(NG):
                    it_lo = io.tile([P, LO_COLS * 8], I16, tag="itlo")
                    nc.sync.dma_start(out=it_lo[:], in_=ins["idx_lo"][g])
                    it_hi = io.tile([P, HI_COLS * 8], I16, tag="ithi")
                    nc.sync.dma_start(out=it_hi[:], in_=ins["idx_hi"][g])
                    dl_t = io.tile([P, KG], BF16, tag="dl")
                    nc.sync.dma_start(out=dl_t[:], in_=ins["dstloc"][g])
                    ea_t = io.tile([EF + 1, KG * P], BF16, tag="ea")
                    nc.sync.dma_start(out=ea_t[:], in_=ins["eaT"][g])
                    gt = gtp.tile([P, KG * Ce], DT, tag="gt")
                    gt3 = gt[:].rearrange("p (n c) -> p n c", c=Ce)
                    gathers(gt3[:, 0:LO_COLS, :], it_lo, tbl[0:SPLIT, :], LO_COLS, Ce)
                    gathers(gt3[:, LO_COLS:KG, :], it_hi, tbl[SPLIT:N, :], HI_COLS, Ce)
                    for b in range(GB):
                        agg_t = aggp.tile([C, DST_BLK], F32, tag="agg")
                        # quads: runs of <=4 chunk-columns, class-pure & contiguous
                        quads = []
                        for c0 in range(0, CBL, 4):
                            quads.append((b * CBL + c0, min(4, CBL - c0)))
                        for c0 in range(0, CBH, 4):
                            quads.append((LO_COLS + b * CBH + c0, min(4, CBH - c0)))
                        ci = 0
                        for (col0, q) in quads:
                            mm = msgp.tile([P, 4 * C], F32, tag="mm")
                            msg_t = sb.tile([P, 4 * C], BF16, tag="msg")
                            if layer == 2:
                                nc.tensor.matmul(mm[:, 0:q * C], lhsT=ident[:],
                                                 rhs=gt[:, col0 * Ce:(col0 + q) * Ce],
                                                 start=True, stop=False,
                                                 skip_group_check=True)
                                for j in range(q):
                                    nc.tensor.matmul(
                                        mm[:, j * C:(j + 1) * C],
                                        lhsT=ea_t[:, (col0 + j) * P:(col0 + j + 1) * P],
                                        rhs=ew[:], start=False, stop=True,
                                        skip_group_check=True)
                                nc.scalar.activation(msg_t[:, 0:q * C],
                                                     mm[:, 0:q * C], AF.Relu)
                            else:
                                for j in range(q):
                                    nc.tensor.matmul(
                                        mm[:, j * C:(j + 1) * C],
                                        lhsT=ea_t[:, (col0 + j) * P:(col0 + j + 1) * P],
                                        rhs=ew[:], start=True, stop=True,
                                        skip_group_check=True)
                                tmp = sb.tile([P, 4 * C], F32, tag="tmp")
                                nc.vector.tensor_tensor(
                                    out=tmp[:, 0:q * C],
                                    in0=gt3[:, col0:col0 + q, 0:NF],
                                    in1=mm[:, 0:q * C], op=ALU.add)
                                nc.scalar.activation(msg_t[:, 0:q * C],
                                                     tmp[:, 0:q * C], AF.Relu)
                            for j in range(q):
                                col = col0 + j
                                oh_t = sb.tile([P, DST_BLK], BF16, tag="oh")
                                nc.vector.tensor_tensor(
                                    out=oh_t[:], in0=iota_b[:],
                                    in1=dl_t[:, col:col + 1].to_broadcast([P, DST_BLK]),
                                    op=ALU.is_equal)
                                nc.tensor.matmul(agg_t[:],
                                                 lhsT=msg_t[:, j * C:(j + 1) * C],
                                                 rhs=oh_t[:],
                                                 start=(ci == 0), stop=(ci == CB - 1))
                                ci += 1
                        colb = (g * GB + b) * DST_BLK
                        dst_sl = slice(colb, colb + DST_BLK)
                        if layer == 1:
                            nc.vector.tensor_tensor(
                                out=hpre[:, dst_sl], in0=xT[:, dst_sl],
                                in1=agg_t[:], op=ALU.add)
                        else:
                            nc.vector.tensor_tensor(
                                out=h2pre[:, dst_sl], in0=hT[:, dst_sl],
                                in1=agg_t[:], op=ALU.add)
                        block_done(g * GB + b, msb, mps)

        def leaky(pool, src_ps, bias, out_ap, width, parts=P):
            t = pool.tile([parts, width], F32, tag="lk")
            nc.scalar.activation(t[:], src_ps[:], AF.Identity, bias=bias[:, 0:1])
            nc.vector.scalar_tensor_tensor(
                out=out_ap, in0=t[:], scalar=0.01, in1=t[:],
                op0=ALU.mult, op1=ALU.max)

        # ---------------- layer 1 ----------------
        def mlp1_block(t, msb, mps):
            s = slice(t * P, (t + 1) * P)
            ps1 = mps.tile([H, P], F32, tag="ps")
            nc.tensor.matmul(ps1[:], lhsT=c1w1[:], rhs=hpre[:, s],
                             start=True, stop=True)
            l1 = msb.tile([H, P], BF16, tag="l1")
            leaky(msb, ps1, c1b1, l1[:], P)
            ps2 = mps.tile([H, P], F32, tag="ps")
            nc.tensor.matmul(ps2[:], lhsT=c1w2[:], rhs=l1[:],
                             start=True, stop=True)
            nc.scalar.activation(hT[:, s], ps2[:], AF.Relu, bias=c1b2[:, 0:1])
            tr = mps.tile([P, H], BF16, tag="ps")
            nc.tensor.transpose(tr[:], hT[:, s], ident[:])
            hrow = msb.tile([P, H], BF16, tag="hrow")
            nc.vector.tensor_copy(out=hrow[:], in_=tr[:])
            nc.sync.dma_start(out=h_shard[s, :], in_=hrow[:])

        edge_phase(1, mlp1_block)

        nc.gpsimd.collective_compute(
            "AllGather", ALU.bypass,
            replica_groups=[list(range(cfg["NCORES"]))],
            ins=[h_shard[:, :]], outs=[h_full[:, :]])

        # ---------------- layer 2 ----------------
        NS_STEP = min(512, NC)
        NS_BLKS = NS_STEP // P

        def mlp2_block(t, msb, mps):
            s = slice(t * P, (t + 1) * P)
            ps1 = mps.tile([H, P], F32, tag="ps")
            nc.tensor.matmul(ps1[:], lhsT=c2w1[:], rhs=h2pre[:, s],
                             start=True, stop=True)
            l1 = msb.tile([H, P], BF16, tag="l1")
            leaky(msb, ps1, c2b1, l1[:], P)
            ps2 = mps.tile([H, P], F32, tag="ps")
            nc.tensor.matmul(ps2[:], lhsT=c2w2[:], rhs=l1[:],
                             start=True, stop=True)
            leaky(msb, ps2, c2b2, h2T[:, s], P)
            ps3 = mps.tile([H, P], F32, tag="ps")
            nc.tensor.matmul(ps3[:], lhsT=nw1[:], rhs=h2T[:, s],
                             start=True, stop=True)
            n1 = msb.tile([H, P], BF16, tag="n1")
            leaky(msb, ps3, nb1, n1[:], P)
            ps4 = mps.tile([H, P], F32, tag="ps")
            nc.tensor.matmul(ps4[:], lhsT=nw2[:], rhs=n1[:],
                             start=True, stop=True)
            leaky(msb, ps4, nb2, nh[:, s], P)
            if (t + 1) % NS_BLKS == 0:
                s0 = (t + 1) * P - NS_STEP
                psn = mps.tile([1, NS_STEP], F32, tag="ps")
                nc.tensor.matmul(psn[:], lhsT=nw3[:], rhs=nh[:, s0:s0 + NS_STEP],
                                 start=True, stop=True)
                nst = msb.tile([1, NS_STEP], F32, tag="nst")
                nc.scalar.activation(nst[:], psn[:], AF.Sigmoid, bias=nb3[:, 0:1])
                nc.sync.dma_start(out=outs["out_ns"][0:1, s0:s0 + NS_STEP], in_=nst[:])

        edge_phase(2, mlp2_block)
        with ExitStack() as mctx:
            msb = mctx.enter_context(tc.tile_pool(name="ahsb", bufs=2))
            mps = mctx.enter_context(tc.tile_pool(name="ahps", bufs=2, space="PSUM"))
            # ---------------- action head ----------------
            pooled = msb.tile([H, GPC], F32, tag="pooled")
            for gi in range(GPC):
                nc.vector.tensor_reduce(
                    out=pooled[:, gi:gi + 1],
                    in_=h2T[:, gi * NPER:(gi + 1) * NPER],
                    axis=mybir.AxisListType.X, op=ALU.add)
            pooled_bf = msb.tile([H, GPC], BF16, tag="pooled_bf")
            nc.vector.tensor_scalar(out=pooled_bf[:], in0=pooled[:],
                                    scalar1=1.0 / NPER, scalar2=None, op0=ALU.mult)
            psa = mps.tile([H, GPC], F32, tag="ps")
            nc.tensor.matmul(psa[:], lhsT=aw1[:], rhs=pooled_bf[:],
                             start=True, stop=True)
            a1 = msb.tile([H, GPC], BF16, tag="a1")
            leaky(msb, psa, ab1, a1[:], GPC)
            psa2 = mps.tile([A, GPC], F32, tag="ps")
            nc.tensor.matmul(psa2[:], lhsT=aw2[:], rhs=a1[:],
                             start=True, stop=True)
            a2 = msb.tile([A, GPC], BF16, tag="a2")
            leaky(msb, psa2, ab2, a2[:], GPC, parts=A)
            trp = mps.tile([GPC, A], BF16, tag="ps")
            nc.tensor.transpose(trp[:], a2[:], ident[:A, :A])
            sm = msb.tile([GPC, A], F32, tag="sm")
            nc.vector.tensor_copy(out=sm[:], in_=trp[:])
            negmax = msb.tile([GPC, 1], F32, tag="negmax")
            nc.vector.tensor_reduce(out=negmax[:], in_=sm[:],
                                    axis=mybir.AxisListType.X, op=ALU.max,
                                    negate=True)
            ex = msb.tile([GPC, A], F32, tag="ex")
            nc.scalar.activation(ex[:], sm[:], AF.Exp, bias=negmax[:, 0:1])
            ssum = msb.tile([GPC, 1], F32, tag="ssum")
            nc.vector.tensor_reduce(out=ssum[:], in_=ex[:],
                                    axis=mybir.AxisListType.X, op=ALU.add)
            rinv = msb.tile([GPC, 1], F32, tag="rinv")
            nc.vector.reciprocal(rinv[:], ssum[:])
            act = msb.tile([GPC, A], F32, tag="act")
            nc.vector.tensor_scalar(out=act[:], in0=ex[:],
                                    scalar1=rinv[:, 0:1], scalar2=None,
                                    op0=ALU.mult)
            nc.sync.dma_start(out=outs["out_act"][:, :], in_=act[:])


# ---------------------------------------------------------------- driver
def build_program(ccfg):
    nc = bacc.Bacc("TRN2", target_bir_lowering=False, debug=False,
                   num_devices=ccfg["NCORES"], num_swdge_queues=NQ)
    N, NF, EF, H, A = ccfg["N"], ccfg["NF"], ccfg["EF"], ccfg["H"], ccfg["A"]
    NC, NG, KG, GPC = ccfg["NC"], ccfg["NG"], ccfg["KG"], ccfg["GPC"]
    DST_BLK, GB = ccfg["DST_BLK"], ccfg["GB"]
    LO_COLS, HI_COLS = GB * ccfg["CBL"], GB * ccfg["CBH"]

    def din(name, shape, dt):
        return nc.dram_tensor(name, list(shape), dt, kind="ExternalInput").ap()

    ins = dict(
        x_pad=din("x_pad", [N, XCOL], F32),
        xT=din("xT", [NF, NC], F32),
        idx_lo=din("idx_lo", [NG, P, LO_COLS * 8], I16),
        idx_hi=din("idx_hi", [NG, P, HI_COLS * 8], I16),
        dstloc=din("dstloc", [NG, P, KG], BF16),
        eaT=din("eaT", [NG, EF + 1, KG * P], BF16),
        ident=din("ident", [P, P], BF16),
        iota_b=din("iota_b", [P, DST_BLK], BF16),
        e1w=din("e1w", [EF + 1, NF], BF16),
        e2w=din("e2w", [EF + 1, H], BF16),
        c1w1=din("c1w1", [NF, H], BF16),
        c1w2=din("c1w2", [H, H], BF16),
        c2w1=din("c2w1", [H, H], BF16),
        c2w2=din("c2w2", [H, H], BF16),
        aw1=din("aw1", [H, H], BF16),
        aw2=din("aw2", [H, A], BF16),
        nw1=din("nw1", [H, H], BF16),
        nw2=din("nw2", [H, H], BF16),
        nw3=din("nw3", [H, 1], BF16),
        c1b1=din("c1b1", [H, 1], F32),
        c1b2=din("c1b2", [H, 1], F32),
        c2b1=din("c2b1", [H, 1], F32),
        c2b2=din("c2b2", [H, 1], F32),
        ab1=din("ab1", [H, 1], F32),
        ab2=din("ab2", [A, 1], F32),
        nb1=din("nb1", [H, 1], F32),
        nb2=din("nb2", [H, 1], F32),
        nb3=din("nb3", [1, 1], F32),
    )
    outs = dict(
        out_act=nc.dram_tensor("out_act", [GPC, A], F32, kind="ExternalOutput").ap(),
        out_ns=nc.dram_tensor("out_ns", [1, NC], F32, kind="ExternalOutput").ap(),
    )
    with tile.TileContext(nc) as tc:
        build_device_kernel(tc, outs, ins, ccfg)
    nc.compile()
    return nc


_PROGRAM_CACHE = {}


def assemble_outputs(cfg, results):
    B, A, NPER = cfg["B"], cfg["A"], cfg["NPER"]
    NCORES = cfg["NCORES"]
    action = np.concatenate([results[k]["out_act"] for k in range(NCORES)], 0)
    ns_all = np.concatenate(
        [results[k]["out_ns"].reshape(-1) for k in range(NCORES)], 0)
    node_scores = np.ascontiguousarray(ns_all.reshape(NPER, B).T)
    return action.astype(np.float32), node_scores.astype(np.float32)


def kernel(**inputs):
    cfg = default_cfg()
    ccfg, in_maps = prep_inputs(cfg, inputs)
    key = (ccfg["CBL"], ccfg["CBH"])
    if key not in _PROGRAM_CACHE:
        _PROGRAM_CACHE[key] = build_program(ccfg)
    nc = _PROGRAM_CACHE[key]
    res = bass_utils.run_bass_kernel_spmd(
        nc, in_maps, core_ids=list(range(ccfg["NCORES"])))
    return assemble_outputs(ccfg, res.results)


# revision 20
# speedup vs baseline: 1.1758x; 1.0545x over previous
# Trainium2 Bass kernel for nn_ActionModel (GINE message passing, 2 layers + heads).
#
# Strategy (8 NeuronCores, SPMD):
#   * Shard edges by dst range: core k owns dst in [k*8192, (k+1)*8192) = 8 graphs.
#   * Host sorts edges by dst, groups them into 128-dst blocks.  Within a block,
#     edges are split into two classes by src < 32768 (dma_gather indices are int16)
#     and each class is padded to a uniform number of 128-edge chunks (CBL/CBH).
#     Every chunk is block-pure and class-pure.
#   * Per-edge x[src] / h[src] rows are fetched with the Ant dma_gather Q7 custom
#     instruction (<=1024 rows per call, round-robin over 4 SWDGE queues so all four
#     Q7 core-pairs generate descriptors in parallel).  The lo/hi class gathers read
#     from the two halves of the table.
#   * Aggregation (segment_sum) is a one-hot matmul on the TensorEngine:
#       aggT[C, 128] += msg[128e, C].T @ onehot[128e, 128d]
#     with onehot generated on the VectorEngine from per-edge local-dst ids (is_equal
#     against a constant iota row).  Padded edges carry dst_local = -1 -> zero column.
#   * Messages: msg = relu(x[src] + edge_attr @ W + b).  edge_attr arrives
#     pre-permuted/transposed with a baked-in ones row so the bias rides in the same
#     matmul.  Layer 2 injects gathered h rows into PSUM with an identity matmul
#     (the add is free on PE); layer 1 adds on the VectorEngine (C=32 is cheap).
#   * After layer-1 local aggregation each core runs the node MLP for ITS 8192 nodes
#     (channel-major), transposes to row-major bf16 and AllGathers h across the 8
#     cores so layer 2's gather reads a device-local table.
#   * Heads: graphs are contiguous 1024-node ranges, fully local to one core.  Host
#     reassembles the [64,10] and [64,1024] outputs.
import os
import sys
import numpy as np

try:
    import ml_dtypes
    _BF16 = ml_dtypes.bfloat16
except ImportError:  # pragma: no cover
    import jax.numpy as jnp
    _BF16 = jnp.bfloat16

for _p in ("/opt/trn_rl_repo",):
    if os.path.isdir(_p) and _p not in sys.path:
        sys.path.append(_p)

import concourse.bass as bass
import concourse.bacc as bacc
import concourse.mybir as mybir
import concourse.tile as tile
from concourse import bass_utils

F32 = mybir.dt.float32
BF16 = mybir.dt.bfloat16
I16 = mybir.dt.int16
AF = mybir.ActivationFunctionType
ALU = mybir.AluOpType
P = 128
XCOL = 64          # x table padded to 64 f32 cols (256B rows for dma_gather)
NQ = 4             # SWDGE queues


def default_cfg():
    return dict(N=65536, E=1048576, NF=32, EF=16, H=128, A=10, B=64,
                NCORES=8, DST_BLK=128, GB=4)


def derive_cfg(cfg, CBL, CBH):
    c = dict(cfg)
    c["CBL"], c["CBH"] = CBL, CBH
    c["CB"] = CBL + CBH               # chunks (of 128 edges) per dst-block
    c["SPLIT"] = c["N"] // 2          # src class threshold (int16 range)
    c["NC"] = c["N"] // c["NCORES"]   # nodes per core
    c["NBLK"] = c["NC"] // c["DST_BLK"]
    c["NG"] = c["NBLK"] // c["GB"]    # groups per core
    c["KG"] = c["GB"] * c["CB"]       # chunk-columns per group
    c["GPC"] = c["B"] // c["NCORES"]  # graphs per core
    c["NPER"] = c["N"] // c["B"]      # nodes per graph
    assert c["NBLK"] % c["GB"] == 0 and c["NC"] % P == 0 and c["NPER"] * c["GPC"] == c["NC"]
    return c


# ---------------------------------------------------------------- host prep
def _wrap_idx16(flat):
    """dma_gather idx layout: [128, S/16] int16, unwrapped[i] = idx[i%16, i//16],
    replicated down the 128 partitions every 16."""
    S = flat.shape[0]
    assert S % 16 == 0
    blk = flat.reshape(S // 16, 16).T        # [16, S/16]
    return np.ascontiguousarray(np.tile(blk, (8, 1))).astype(np.int16)


def prep_inputs(cfg, inputs):
    """Shard + sort + pad on host.  Returns (ccfg, in_maps)."""
    N, E = cfg["N"], cfg["E"]
    NF, EF, H = cfg["NF"], cfg["EF"], cfg["H"]
    NCORES, DST_BLK, GB = cfg["NCORES"], cfg["DST_BLK"], cfg["GB"]
    NC = N // NCORES
    NBLK = NC // DST_BLK
    SPLIT = N // 2

    x = np.asarray(inputs["x"], np.float32)
    ei = np.asarray(inputs["edge_index"])
    ea = np.asarray(inputs["edge_attr"], np.float32)
    src_all = ei[0].astype(np.int64)
    dst_all = ei[1].astype(np.int64)

    cores = []
    max_lo = max_hi = 0
    for k in range(NCORES):
        m = (dst_all >= k * NC) & (dst_all < (k + 1) * NC)
        eidx = np.nonzero(m)[0]
        dl = dst_all[eidx] - k * NC
        cls = (src_all[eidx] >= SPLIT).astype(np.int64)
        blk = dl // DST_BLK
        order = np.lexsort((cls, blk))
        eidx, dl, cls, blk = eidx[order], dl[order], cls[order], blk[order]
        cnt_lo = np.bincount(blk[cls == 0], minlength=NBLK)
        cnt_hi = np.bincount(blk[cls == 1], minlength=NBLK)
        max_lo = max(max_lo, int(cnt_lo.max()))
        max_hi = max(max_hi, int(cnt_hi.max()))
        cores.append((eidx, dl, cls, blk, cnt_lo, cnt_hi))

    CBL = max(2, -(-max_lo // P))
    CBH = max(2, -(-max_hi // P))
    ccfg = derive_cfg(cfg, CBL, CBH)
    NG, KG, CB = ccfg["NG"], ccfg["KG"], ccfg["CB"]
    LO_COLS, HI_COLS = GB * CBL, GB * CBH

    ident = np.eye(P, dtype=_BF16)
    iota_b = np.ascontiguousarray(
        np.broadcast_to(np.arange(DST_BLK, dtype=np.float32), (P, DST_BLK))).astype(_BF16)
    x_pad = np.zeros((N, XCOL), np.float32)
    x_pad[:, :NF] = x

    def bf(a):
        return np.asarray(a, np.float32).astype(_BF16)

    def b2(a, d):
        return np.asarray(a, np.float32).reshape(d, 1)

    shared = dict(
        x_pad=x_pad, ident=ident, iota_b=iota_b,
        e1w=np.concatenate([np.asarray(inputs["e1_w"], np.float32),
                            np.asarray(inputs["e1_b"], np.float32)[None]], 0).astype(_BF16),
        e2w=np.concatenate([np.asarray(inputs["e2_w"], np.float32),
                            np.asarray(inputs["e2_b"], np.float32)[None]], 0).astype(_BF16),
        c1w1=bf(inputs["c1_w1"]), c1w2=bf(inputs["c1_w2"]),
        c2w1=bf(inputs["c2_w1"]), c2w2=bf(inputs["c2_w2"]),
        aw1=bf(inputs["a_w1"]), aw2=bf(inputs["a_w2"]),
        nw1=bf(inputs["n_w1"]), nw2=bf(inputs["n_w2"]), nw3=bf(inputs["n_w3"]),
        c1b1=b2(inputs["c1_b1"], H), c1b2=b2(inputs["c1_b2"], H),
        c2b1=b2(inputs["c2_b1"], H), c2b2=b2(inputs["c2_b2"], H),
        ab1=b2(inputs["a_b1"], H), ab2=b2(inputs["a_b2"], cfg["A"]),
        nb1=b2(inputs["n_b1"], H), nb2=b2(inputs["n_b2"], H),
        nb3=b2(inputs["n_b3"], 1),
    )

    in_maps = []
    for k in range(NCORES):
        eidx, dl, cls, blk, cnt_lo, cnt_hi = cores[k]
        starts_lo = np.zeros(NBLK + 1, np.int64); np.cumsum(cnt_lo, out=starts_lo[1:])
        starts_hi = np.zeros(NBLK + 1, np.int64); np.cumsum(cnt_hi, out=starts_hi[1:])
        # within (block, class) ranks: edges are sorted by (blk, cls)
        r = np.arange(len(dl))
        base = np.where(cls == 0,
                        starts_lo[blk] + starts_hi[blk],
                        starts_lo[blk + 1] + starts_hi[blk])
        r_in = r - base
        g = blk // GB
        b = blk % GB
        ci = r_in >> 7
        p = r_in & 127
        col = np.where(cls == 0, b * CBL + ci, LO_COLS + b * CBH + ci)

        dstloc = np.full((NG, P, KG), -1.0, np.float32)  # cast to bf16 below
        eaT = np.zeros((NG, EF + 1, KG * P), np.float32)
        eaT[:, EF, :] = 1.0
        dstloc[g, p, col] = (dl % DST_BLK).astype(np.float32)
        eaT[g.reshape(-1, 1), np.arange(EF).reshape(1, -1),
            (col * P + p).reshape(-1, 1)] = ea[eidx]

        idx_lo = np.zeros((NG, LO_COLS * P), np.int64)
        idx_hi = np.zeros((NG, HI_COLS * P), np.int64)
        lo_m, hi_m = cls == 0, cls == 1
        idx_lo[g[lo_m], (col[lo_m]) * P + p[lo_m]] = src_all[eidx[lo_m]]
        idx_hi[g[hi_m], (col[hi_m] - LO_COLS) * P + p[hi_m]] = src_all[eidx[hi_m]] - SPLIT
        im = dict(shared)
        im["idx_lo"] = np.stack([_wrap_idx16(idx_lo[gg]) for gg in range(NG)])
        im["idx_hi"] = np.stack([_wrap_idx16(idx_hi[gg]) for gg in range(NG)])
        im["dstloc"] = dstloc.astype(_BF16)
        im["eaT"] = eaT.astype(_BF16)
        im["xT"] = np.ascontiguousarray(x[k * NC:(k + 1) * NC].T)
        in_maps.append(im)
    return ccfg, in_maps


# ---------------------------------------------------------------- device code
def build_device_kernel(tc, outs, ins, cfg):
    nc = tc.nc
    N, NF, EF, H, A = cfg["N"], cfg["NF"], cfg["EF"], cfg["H"], cfg["A"]
    NC, NG, GB, KG = cfg["NC"], cfg["NG"], cfg["GB"], cfg["KG"]
    CBL, CBH, CB = cfg["CBL"], cfg["CBH"], cfg["CB"]
    DST_BLK, GPC, NPER, SPLIT = cfg["DST_BLK"], cfg["GPC"], cfg["NPER"], cfg["SPLIT"]
    NTILE = NC // P
    LO_COLS, HI_COLS = GB * CBL, GB * CBH
    qctr = [0]

    from contextlib import ExitStack
    with ExitStack() as ctx:
        const = ctx.enter_context(tc.tile_pool(name="const", bufs=1))
        big = ctx.enter_context(tc.tile_pool(name="big", bufs=1))
        dram = ctx.enter_context(tc.tile_pool(name="dram", bufs=1, space="DRAM"))

        def load_const(name, shape, dtype):
            t = const.tile(list(shape), dtype, tag=name)
            nc.sync.dma_start(out=t[:], in_=ins[name][:])
            return t

        ident = load_const("ident", [P, P], BF16)
        iota_b = load_const("iota_b", [P, DST_BLK], BF16)
        e1w = load_const("e1w", [EF + 1, NF], BF16)
        e2w = load_const("e2w", [EF + 1, H], BF16)
        c1w1 = load_const("c1w1", [NF, H], BF16)
        c1w2 = load_const("c1w2", [H, H], BF16)
        c2w1 = load_const("c2w1", [H, H], BF16)
        c2w2 = load_const("c2w2", [H, H], BF16)
        aw1 = load_const("aw1", [H, H], BF16)
        aw2 = load_const("aw2", [H, A], BF16)
        nw1 = load_const("nw1", [H, H], BF16)
        nw2 = load_const("nw2", [H, H], BF16)
        nw3 = load_const("nw3", [H, 1], BF16)
        c1b1 = load_const("c1b1", [H, 1], F32)
        c1b2 = load_const("c1b2", [H, 1], F32)
        c2b1 = load_const("c2b1", [H, 1], F32)
        c2b2 = load_const("c2b2", [H, 1], F32)
        ab1 = load_const("ab1", [H, 1], F32)
        ab2 = load_const("ab2", [A, 1], F32)
        nb1 = load_const("nb1", [H, 1], F32)
        nb2 = load_const("nb2", [H, 1], F32)
        nb3 = load_const("nb3", [1, 1], F32)
        xT = load_const("xT", [NF, NC], F32)

        hpre = big.tile([NF, NC], BF16, tag="hpre")    # x + agg1 (channel-major)
        hT = big.tile([H, NC], BF16, tag="hT")         # relu(mlp1) local, channel-major
        h2pre = big.tile([H, NC], BF16, tag="h2pre")
        h2T = big.tile([H, NC], BF16, tag="h2T")
        nh = big.tile([H, NC], BF16, tag="nh")

        h_shard = dram.tile([NC, H], BF16, tag="h_shard")
        h_full = dram.tile([N, H], BF16, tag="h_full", addr_space="Shared")

        def gathers(gt3, it, table, ncols, Ce):
            """Issue <=1024-row dma_gather calls covering ncols chunk-columns."""
            off = 0
            while off < ncols:
                n = min(8, ncols - off)
                ni = n * P
                nc.gpsimd.dma_gather(
                    out_ap=gt3[:, off:off + n, :],
                    in_ap=table,
                    idxs_ap=it[:, off * 8:off * 8 + n * 8],
                    num_idxs=ni, num_idxs_reg=ni, elem_size=Ce,
                    queue_num=qctr[0] % NQ)
                qctr[0] += 1
                off += n

        def edge_phase(layer, block_done):
            C = NF if layer == 1 else H
            Ce = XCOL if layer == 1 else H          # gathered row width
            DT = F32 if layer == 1 else BF16
            ew = e1w if layer == 1 else e2w
            tbl = ins["x_pad"] if layer == 1 else h_full
            with ExitStack() as ectx:
                io = ectx.enter_context(tc.tile_pool(name=f"io{layer}", bufs=2))
                gtp = ectx.enter_context(tc.tile_pool(name=f"gtp{layer}", bufs=2))
                sb = ectx.enter_context(tc.tile_pool(name=f"sb{layer}", bufs=3))
                msgp = ectx.enter_context(
                    tc.tile_pool(name=f"msgp{layer}", bufs=3, space="PSUM"))
                aggp = ectx.enter_context(
                    tc.tile_pool(name=f"aggp{layer}", bufs=3, space="PSUM"))
                msb = ectx.enter_context(tc.tile_pool(name=f"m{layer}sb", bufs=3))
                mps = ectx.enter_context(
                    tc.tile_pool(name=f"m{layer}ps", bufs=2, space="PSUM"))
                for g in range(NG):
                    it_lo = io.tile([P, LO_COLS * 8], I16, tag="itlo")
                    nc.sync.dma_start(out=it_lo[:], in_=ins["idx_lo"][g])
                    it_hi = io.tile([P, HI_COLS * 8], I16, tag="ithi")
                    nc.sync.dma_start(out=it_hi[:], in_=ins["idx_hi"][g])
                    dl_t = io.tile([P, KG], BF16, tag="dl")
                    nc.sync.dma_start(out=dl_t[:], in_=ins["dstloc"][g])
                    ea_t = io.tile([EF + 1, KG * P], BF16, tag="ea")
                    nc.sync.dma_start(out=ea_t[:], in_=ins["eaT"][g])
                    gt = gtp.tile([P, KG * Ce], DT, tag="gt")
                    gt3 = gt[:].rearrange("p (n c) -> p n c", c=Ce)
                    gathers(gt3[:, 0:LO_COLS, :], it_lo, tbl[0:SPLIT, :], LO_COLS, Ce)
                    gathers(gt3[:, LO_COLS:KG, :], it_hi, tbl[SPLIT:N, :], HI_COLS, Ce)
                    for b in range(GB):
                        agg_t = aggp.tile([C, DST_BLK], F32, tag="agg")
                        # quads: runs of <=4 chunk-columns, class-pure & contiguous
                        quads = []
                        for c0 in range(0, CBL, 4):
                            quads.append((b * CBL + c0, min(4, CBL - c0)))
                        for c0 in range(0, CBH, 4):
                            quads.append((LO_COLS + b * CBH + c0, min(4, CBH - c0)))
                        ci = 0
                        for (col0, q) in quads:
                            mm = msgp.tile([P, 4 * C], F32, tag="mm")
                            msg_t = sb.tile([P, 4 * C], BF16, tag="msg")
                            if layer == 2:
                                nc.tensor.matmul(mm[:, 0:q * C], lhsT=ident[:],
                                                 rhs=gt[:, col0 * Ce:(col0 + q) * Ce],
                                                 start=True, stop=False,
                                                 skip_group_check=True)
                                for j in range(q):
                                    nc.tensor.matmul(
                                        mm[:, j * C:(j + 1) * C],
                                        lhsT=ea_t[:, (col0 + j) * P:(col0 + j + 1) * P],
                                        rhs=ew[:], start=False, stop=True,
                                        skip_group_check=True)
                                nc.scalar.activation(msg_t[:, 0:q * C],
                                                     mm[:, 0:q * C], AF.Relu)
                            else:
                                for j in range(q):
                                    nc.tensor.matmul(
                                        mm[:, j * C:(j + 1) * C],
                                        lhsT=ea_t[:, (col0 + j) * P:(col0 + j + 1) * P],
                                        rhs=ew[:], start=True, stop=True,
                                        skip_group_check=True)
                                tmp = sb.tile([P, 4 * C], F32, tag="tmp")
                                nc.vector.tensor_tensor(
                                    out=tmp[:, 0:q * C],
                                    in0=gt3[:, col0:col0 + q, 0:NF],
                                    in1=mm[:, 0:q * C], op=ALU.add)
                                nc.scalar.activation(msg_t[:, 0:q * C],
                                                     tmp[:, 0:q * C], AF.Relu)
                            for j in range(q):
                                col = col0 + j
                                oh_t = sb.tile([P, DST_BLK], BF16, tag="oh")
                                nc.vector.tensor_tensor(
                                    out=oh_t[:], in0=iota_b[:],
                                    in1=dl_t[:, col:col + 1].to_broadcast([P, DST_BLK]),
                                    op=ALU.is_equal)
                                nc.tensor.matmul(agg_t[:],
                                                 lhsT=msg_t[:, j * C:(j + 1) * C],
                                                 rhs=oh_t[:],
                                                 start=(ci == 0), stop=(ci == CB - 1))
                                ci += 1
                        colb = (g * GB + b) * DST_BLK
                        dst_sl = slice(colb, colb + DST_BLK)
                        if layer == 1:
                            nc.vector.tensor_tensor(
                                out=hpre[:, dst_sl], in0=xT[:, dst_sl],
                                in1=agg_t[:], op=ALU.add)
                        else:
                            nc.vector.tensor_tensor(
                                out=h2pre[:, dst_sl], in0=hT[:, dst_sl],
                                in1=agg_t[:], op=ALU.add)
                        block_done(g * GB + b, msb, mps)

        def leaky(pool, src_ps, bias, out_ap, width, parts=P):
            nc.scalar.activation(out_ap, src_ps[:], AF.Prelu,
                                 bias=bias[:, 0:1], alpha=0.01)

        # ---------------- layer 1 ----------------
        def mlp1_block(t, msb, mps):
            s = slice(t * P, (t + 1) * P)
            ps1 = mps.tile([H, P], F32, tag="ps")
            nc.tensor.matmul(ps1[:], lhsT=c1w1[:], rhs=hpre[:, s],
                             start=True, stop=True)
            l1 = msb.tile([H, P], BF16, tag="l1")
            leaky(msb, ps1, c1b1, l1[:], P)
            ps2 = mps.tile([H, P], F32, tag="ps")
            nc.tensor.matmul(ps2[:], lhsT=c1w2[:], rhs=l1[:],
                             start=True, stop=True)
            nc.scalar.activation(hT[:, s], ps2[:], AF.Relu, bias=c1b2[:, 0:1])
            tr = mps.tile([P, H], BF16, tag="ps")
            nc.tensor.transpose(tr[:], hT[:, s], ident[:])
            hrow = msb.tile([P, H], BF16, tag="hrow")
            nc.vector.tensor_copy(out=hrow[:], in_=tr[:])
            nc.sync.dma_start(out=h_shard[s, :], in_=hrow[:])

        edge_phase(1, mlp1_block)

        nc.gpsimd.collective_compute(
            "AllGather", ALU.bypass,
            replica_groups=[list(range(cfg["NCORES"]))],
            ins=[h_shard[:, :]], outs=[h_full[:, :]])

        # ---------------- layer 2 ----------------
        NS_STEP = min(512, NC)
        NS_BLKS = NS_STEP // P

        def mlp2_block(t, msb, mps):
            s = slice(t * P, (t + 1) * P)
            ps1 = mps.tile([H, P], F32, tag="ps")
            nc.tensor.matmul(ps1[:], lhsT=c2w1[:], rhs=h2pre[:, s],
                             start=True, stop=True)
            l1 = msb.tile([H, P], BF16, tag="l1")
            leaky(msb, ps1, c2b1, l1[:], P)
            ps2 = mps.tile([H, P], F32, tag="ps")
            nc.tensor.matmul(ps2[:], lhsT=c2w2[:], rhs=l1[:],
                             start=True, stop=True)
            leaky(msb, ps2, c2b2, h2T[:, s], P)
            ps3 = mps.tile([H, P], F32, tag="ps")
            nc.tensor.matmul(ps3[:], lhsT=nw1[:], rhs=h2T[:, s],
                             start=True, stop=True)
            n1 = msb.tile([H, P], BF16, tag="n1")
            leaky(msb, ps3, nb1, n1[:], P)
            ps4 = mps.tile([H, P], F32, tag="ps")
            nc.tensor.matmul(ps4[:], lhsT=nw2[:], rhs=n1[:],
                             start=True, stop=True)
            leaky(msb, ps4, nb2, nh[:, s], P)
            if (t + 1) % NS_BLKS == 0:
                s0 = (t + 1) * P - NS_STEP
                psn = mps.tile([1, NS_STEP], F32, tag="ps")
                nc.tensor.matmul(psn[:], lhsT=nw3[:], rhs=nh[:, s0:s0 + NS_STEP],
                                 start=True, stop=True)
                nst = msb.tile([1, NS_STEP], F32, tag="nst")
                nc.scalar.activation(nst[:], psn[:], AF.Sigmoid, bias=nb3[:, 0:1])
                nc.sync.dma_start(out=outs["out_ns"][0:1, s0:s0 + NS_STEP], in_=nst[:])

        edge_phase(2, mlp2_block)
        with ExitStack() as mctx:
            msb = mctx.enter_context(tc.tile_pool(name="ahsb", bufs=2))
            mps = mctx.enter_context(tc.tile_pool(name="ahps", bufs=2, space="PSUM"))
            # ---------------- action head ----------------
            pooled = msb.tile([H, GPC], F32, tag="pooled")
            for gi in range(GPC):
                nc.vector.tensor_reduce(
                    out=pooled[:, gi:gi + 1],
                    in_=h2T[:, gi * NPER:(gi + 1) * NPER],
                    axis=mybir.AxisListType.X, op=ALU.add)
            pooled_bf = msb.tile([H, GPC], BF16, tag="pooled_bf")
            nc.vector.tensor_scalar(out=pooled_bf[:], in0=pooled[:],
                                    scalar1=1.0 / NPER, scalar2=None, op0=ALU.mult)
            psa = mps.tile([H, GPC], F32, tag="ps")
            nc.tensor.matmul(psa[:], lhsT=aw1[:], rhs=pooled_bf[:],
                             start=True, stop=True)
            a1 = msb.tile([H, GPC], BF16, tag="a1")
            leaky(msb, psa, ab1, a1[:], GPC)
            psa2 = mps.tile([A, GPC], F32, tag="ps")
            nc.tensor.matmul(psa2[:], lhsT=aw2[:], rhs=a1[:],
                             start=True, stop=True)
            a2 = msb.tile([A, GPC], BF16, tag="a2")
            leaky(msb, psa2, ab2, a2[:], GPC, parts=A)
            trp = mps.tile([GPC, A], BF16, tag="ps")
            nc.tensor.transpose(trp[:], a2[:], ident[:A, :A])
            sm = msb.tile([GPC, A], F32, tag="sm")
            nc.vector.tensor_copy(out=sm[:], in_=trp[:])
            negmax = msb.tile([GPC, 1], F32, tag="negmax")
            nc.vector.tensor_reduce(out=negmax[:], in_=sm[:],
                                    axis=mybir.AxisListType.X, op=ALU.max,
                                    negate=True)
            ex = msb.tile([GPC, A], F32, tag="ex")
            nc.scalar.activation(ex[:], sm[:], AF.Exp, bias=negmax[:, 0:1])
            ssum = msb.tile([GPC, 1], F32, tag="ssum")
            nc.vector.tensor_reduce(out=ssum[:], in_=ex[:],
                                    axis=mybir.AxisListType.X, op=ALU.add)
            rinv = msb.tile([GPC, 1], F32, tag="rinv")
            nc.vector.reciprocal(rinv[:], ssum[:])
            act = msb.tile([GPC, A], F32, tag="act")
            nc.vector.tensor_scalar(out=act[:], in0=ex[:],
                                    scalar1=rinv[:, 0:1], scalar2=None,
                                    op0=ALU.mult)
            nc.sync.dma_start(out=outs["out_act"][:, :], in_=act[:])


# ---------------------------------------------------------------- driver
def build_program(ccfg):
    nc = bacc.Bacc("TRN2", target_bir_lowering=False, debug=False,
                   num_devices=ccfg["NCORES"], num_swdge_queues=NQ)
    N, NF, EF, H, A = ccfg["N"], ccfg["NF"], ccfg["EF"], ccfg["H"], ccfg["A"]
    NC, NG, KG, GPC = ccfg["NC"], ccfg["NG"], ccfg["KG"], ccfg["GPC"]
    DST_BLK, GB = ccfg["DST_BLK"], ccfg["GB"]
    LO_COLS, HI_COLS = GB * ccfg["CBL"], GB * ccfg["CBH"]

    def din(name, shape, dt):
        return nc.dram_tensor(name, list(shape), dt, kind="ExternalInput").ap()

    ins = dict(
        x_pad=din("x_pad", [N, XCOL], F32),
        xT=din("xT", [NF, NC], F32),
        idx_lo=din("idx_lo", [NG, P, LO_COLS * 8], I16),
        idx_hi=din("idx_hi", [NG, P, HI_COLS * 8], I16),
        dstloc=din("dstloc", [NG, P, KG], BF16),
        eaT=din("eaT", [NG, EF + 1, KG * P], BF16),
        ident=din("ident", [P, P], BF16),
        iota_b=din("iota_b", [P, DST_BLK], BF16),
        e1w=din("e1w", [EF + 1, NF], BF16),
        e2w=din("e2w", [EF + 1, H], BF16),
        c1w1=din("c1w1", [NF, H], BF16),
        c1w2=din("c1w2", [H, H], BF16),
        c2w1=din("c2w1", [H, H], BF16),
        c2w2=din("c2w2", [H, H], BF16),
        aw1=din("aw1", [H, H], BF16),
        aw2=din("aw2", [H, A], BF16),
        nw1=din("nw1", [H, H], BF16),
        nw2=din("nw2", [H, H], BF16),
        nw3=din("nw3", [H, 1], BF16),
        c1b1=din("c1b1", [H, 1], F32),
        c1b2=din("c1b2", [H, 1], F32),
        c2b1=din("c2b1", [H, 1], F32),
        c2b2=din("c2b2", [H, 1], F32),
        ab1=din("ab1", [H, 1], F32),
        ab2=din("ab2", [A, 1], F32),
        nb1=din("nb1", [H, 1], F32),
        nb2=din("nb2", [H, 1], F32),
        nb3=din("nb3", [1, 1], F32),
    )
    outs = dict(
        out_act=nc.dram_tensor("out_act", [GPC, A], F32, kind="ExternalOutput").ap(),
        out_ns=nc.dram_tensor("out_ns", [1, NC], F32, kind="ExternalOutput").ap(),
    )
    with tile.TileContext(nc) as tc:
        build_device_kernel(tc, outs, ins, ccfg)
    nc.compile()
    return nc


_PROGRAM_CACHE = {}


def assemble_outputs(cfg, results):
    B, A, NPER = cfg["B"], cfg["A"], cfg["NPER"]
    NCORES = cfg["NCORES"]
    action = np.concatenate([results[k]["out_act"] for k in range(NCORES)], 0)
    ns_all = np.concatenate(
        [results[k]["out_ns"].reshape(-1) for k in range(NCORES)], 0)
    node_scores = np.ascontiguousarray(ns_all.reshape(NPER, B).T)
    return action.astype(np.float32), node_scores.astype(np.float32)


def kernel(**inputs):
    cfg = default_cfg()
    ccfg, in_maps = prep_inputs(cfg, inputs)
    key = (ccfg["CBL"], ccfg["CBH"])
    if key not in _PROGRAM_CACHE:
        _PROGRAM_CACHE[key] = build_program(ccfg)
    nc = _PROGRAM_CACHE[key]
    res = bass_utils.run_bass_kernel_spmd(
        nc, in_maps, core_ids=list(range(ccfg["NCORES"])))
    return assemble_outputs(ccfg, res.results)


# revision 22
# speedup vs baseline: 1.2673x; 1.0779x over previous
# Trainium2 Bass kernel for nn_ActionModel (GINE message passing, 2 layers + heads).
#
# Strategy (8 NeuronCores, SPMD):
#   * Shard edges by dst range: core k owns dst in [k*8192, (k+1)*8192) = 8 graphs.
#   * Host sorts edges by dst, groups them into 128-dst blocks.  Within a block,
#     edges are split into two classes by src < 32768 (dma_gather indices are int16)
#     and each class is padded to a uniform number of 128-edge chunks (CBL/CBH).
#     Every chunk is block-pure and class-pure.
#   * Per-edge x[src] / h[src] rows are fetched with the Ant dma_gather Q7 custom
#     instruction (<=1024 rows per call, round-robin over 4 SWDGE queues so all four
#     Q7 core-pairs generate descriptors in parallel).  The lo/hi class gathers read
#     from the two halves of the table.
#   * Aggregation (segment_sum) is a one-hot matmul on the TensorEngine:
#       aggT[C, 128] += msg[128e, C].T @ onehot[128e, 128d]
#     with onehot generated on the VectorEngine from per-edge local-dst ids (is_equal
#     against a constant iota row).  Padded edges carry dst_local = -1 -> zero column.
#   * Messages: msg = relu(x[src] + edge_attr @ W + b).  edge_attr arrives
#     pre-permuted/transposed with a baked-in ones row so the bias rides in the same
#     matmul.  Layer 2 injects gathered h rows into PSUM with an identity matmul
#     (the add is free on PE); layer 1 adds on the VectorEngine (C=32 is cheap).
#   * After layer-1 local aggregation each core runs the node MLP for ITS 8192 nodes
#     (channel-major), transposes to row-major bf16 and AllGathers h across the 8
#     cores so layer 2's gather reads a device-local table.
#   * Heads: graphs are contiguous 1024-node ranges, fully local to one core.  Host
#     reassembles the [64,10] and [64,1024] outputs.
import os
import sys
import numpy as np

try:
    import ml_dtypes
    _BF16 = ml_dtypes.bfloat16
except ImportError:  # pragma: no cover
    import jax.numpy as jnp
    _BF16 = jnp.bfloat16

for _p in ("/opt/trn_rl_repo",):
    if os.path.isdir(_p) and _p not in sys.path:
        sys.path.append(_p)

import concourse.bass as bass
import concourse.bacc as bacc
import concourse.mybir as mybir
import concourse.tile as tile
from concourse import bass_utils

F32 = mybir.dt.float32
BF16 = mybir.dt.bfloat16
I16 = mybir.dt.int16
AF = mybir.ActivationFunctionType
ALU = mybir.AluOpType
P = 128
XCOL = 64          # x table padded to 64 f32 cols (256B rows for dma_gather)
NQ = 4             # SWDGE queues


def default_cfg():
    return dict(N=65536, E=1048576, NF=32, EF=16, H=128, A=10, B=64,
                NCORES=8, DST_BLK=128, GB=4)


def derive_cfg(cfg, CBL, CBH):
    c = dict(cfg)
    c["CBL"], c["CBH"] = CBL, CBH
    c["CB"] = CBL + CBH               # chunks (of 128 edges) per dst-block
    c["SPLIT"] = c["N"] // 2          # src class threshold (int16 range)
    c["NC"] = c["N"] // c["NCORES"]   # nodes per core
    c["NBLK"] = c["NC"] // c["DST_BLK"]
    c["NG"] = c["NBLK"] // c["GB"]    # groups per core
    c["KG"] = c["GB"] * c["CB"]       # chunk-columns per group
    c["GPC"] = c["B"] // c["NCORES"]  # graphs per core
    c["NPER"] = c["N"] // c["B"]      # nodes per graph
    assert c["NBLK"] % c["GB"] == 0 and c["NC"] % P == 0 and c["NPER"] * c["GPC"] == c["NC"]
    return c


# ---------------------------------------------------------------- host prep
def _wrap_idx16(flat):
    """dma_gather idx layout: [128, S/16] int16, unwrapped[i] = idx[i%16, i//16],
    replicated down the 128 partitions every 16."""
    S = flat.shape[0]
    assert S % 16 == 0
    blk = flat.reshape(S // 16, 16).T        # [16, S/16]
    return np.ascontiguousarray(np.tile(blk, (8, 1))).astype(np.int16)


def prep_inputs(cfg, inputs):
    """Shard + sort + pad on host.  Returns (ccfg, in_maps)."""
    N, E = cfg["N"], cfg["E"]
    NF, EF, H = cfg["NF"], cfg["EF"], cfg["H"]
    NCORES, DST_BLK, GB = cfg["NCORES"], cfg["DST_BLK"], cfg["GB"]
    NC = N // NCORES
    NBLK = NC // DST_BLK
    SPLIT = N // 2

    x = np.asarray(inputs["x"], np.float32)
    ei = np.asarray(inputs["edge_index"])
    ea = np.asarray(inputs["edge_attr"], np.float32)
    src_all = ei[0].astype(np.int64)
    dst_all = ei[1].astype(np.int64)

    cores = []
    max_lo = max_hi = 0
    for k in range(NCORES):
        m = (dst_all >= k * NC) & (dst_all < (k + 1) * NC)
        eidx = np.nonzero(m)[0]
        dl = dst_all[eidx] - k * NC
        cls = (src_all[eidx] >= SPLIT).astype(np.int64)
        blk = dl // DST_BLK
        order = np.lexsort((cls, blk))
        eidx, dl, cls, blk = eidx[order], dl[order], cls[order], blk[order]
        cnt_lo = np.bincount(blk[cls == 0], minlength=NBLK)
        cnt_hi = np.bincount(blk[cls == 1], minlength=NBLK)
        max_lo = max(max_lo, int(cnt_lo.max()))
        max_hi = max(max_hi, int(cnt_hi.max()))
        cores.append((eidx, dl, cls, blk, cnt_lo, cnt_hi))

    CBL = max(2, -(-max_lo // P))
    CBH = max(2, -(-max_hi // P))
    ccfg = derive_cfg(cfg, CBL, CBH)
    NG, KG, CB = ccfg["NG"], ccfg["KG"], ccfg["CB"]
    LO_COLS, HI_COLS = GB * CBL, GB * CBH

    ident = np.eye(P, dtype=_BF16)
    iota_b = np.ascontiguousarray(
        np.broadcast_to(np.arange(DST_BLK, dtype=np.float32), (P, DST_BLK))).astype(_BF16)
    x_pad = np.zeros((N, XCOL), np.float32)
    x_pad[:, :NF] = x

    def bf(a):
        return np.asarray(a, np.float32).astype(_BF16)

    def b2(a, d):
        return np.asarray(a, np.float32).reshape(d, 1)

    shared = dict(
        x_pad=x_pad, ident=ident, iota_b=iota_b,
        e1w=np.concatenate([np.asarray(inputs["e1_w"], np.float32),
                            np.asarray(inputs["e1_b"], np.float32)[None]], 0).astype(_BF16),
        e2w=np.concatenate([np.asarray(inputs["e2_w"], np.float32),
                            np.asarray(inputs["e2_b"], np.float32)[None]], 0).astype(_BF16),
        c1w1=bf(inputs["c1_w1"]), c1w2=bf(inputs["c1_w2"]),
        c2w1=bf(inputs["c2_w1"]), c2w2=bf(inputs["c2_w2"]),
        aw1=bf(inputs["a_w1"]), aw2=bf(inputs["a_w2"]),
        nw1=bf(inputs["n_w1"]), nw2=bf(inputs["n_w2"]), nw3=bf(inputs["n_w3"]),
        c1b1=b2(inputs["c1_b1"], H), c1b2=b2(inputs["c1_b2"], H),
        c2b1=b2(inputs["c2_b1"], H), c2b2=b2(inputs["c2_b2"], H),
        ab1=b2(inputs["a_b1"], H), ab2=b2(inputs["a_b2"], cfg["A"]),
        nb1=b2(inputs["n_b1"], H), nb2=b2(inputs["n_b2"], H),
        nb3=b2(inputs["n_b3"], 1),
    )

    in_maps = []
    for k in range(NCORES):
        eidx, dl, cls, blk, cnt_lo, cnt_hi = cores[k]
        starts_lo = np.zeros(NBLK + 1, np.int64); np.cumsum(cnt_lo, out=starts_lo[1:])
        starts_hi = np.zeros(NBLK + 1, np.int64); np.cumsum(cnt_hi, out=starts_hi[1:])
        # within (block, class) ranks: edges are sorted by (blk, cls)
        r = np.arange(len(dl))
        base = np.where(cls == 0,
                        starts_lo[blk] + starts_hi[blk],
                        starts_lo[blk + 1] + starts_hi[blk])
        r_in = r - base
        g = blk // GB
        b = blk % GB
        ci = r_in >> 7
        p = r_in & 127
        col = np.where(cls == 0, b * CBL + ci, LO_COLS + b * CBH + ci)

        dstloc = np.full((NG, P, KG), -1.0, np.float32)  # cast to bf16 below
        eaT = np.zeros((NG, EF + 1, KG * P), np.float32)
        eaT[:, EF, :] = 1.0
        dstloc[g, p, col] = (dl % DST_BLK).astype(np.float32)
        eaT[g.reshape(-1, 1), np.arange(EF).reshape(1, -1),
            (col * P + p).reshape(-1, 1)] = ea[eidx]

        idx_lo = np.zeros((NG, LO_COLS * P), np.int64)
        idx_hi = np.zeros((NG, HI_COLS * P), np.int64)
        lo_m, hi_m = cls == 0, cls == 1
        idx_lo[g[lo_m], (col[lo_m]) * P + p[lo_m]] = src_all[eidx[lo_m]]
        idx_hi[g[hi_m], (col[hi_m] - LO_COLS) * P + p[hi_m]] = src_all[eidx[hi_m]] - SPLIT
        im = dict(shared)
        im["idx_lo"] = np.stack([_wrap_idx16(idx_lo[gg]) for gg in range(NG)])
        im["idx_hi"] = np.stack([_wrap_idx16(idx_hi[gg]) for gg in range(NG)])
        im["dstloc"] = dstloc.astype(_BF16)
        im["eaT"] = eaT.astype(_BF16)
        im["xT"] = np.ascontiguousarray(x[k * NC:(k + 1) * NC].T)
        in_maps.append(im)
    return ccfg, in_maps


# ---------------------------------------------------------------- device code
def build_device_kernel(tc, outs, ins, cfg):
    nc = tc.nc
    N, NF, EF, H, A = cfg["N"], cfg["NF"], cfg["EF"], cfg["H"], cfg["A"]
    NC, NG, GB, KG = cfg["NC"], cfg["NG"], cfg["GB"], cfg["KG"]
    CBL, CBH, CB = cfg["CBL"], cfg["CBH"], cfg["CB"]
    DST_BLK, GPC, NPER, SPLIT = cfg["DST_BLK"], cfg["GPC"], cfg["NPER"], cfg["SPLIT"]
    NTILE = NC // P
    LO_COLS, HI_COLS = GB * CBL, GB * CBH
    qctr = [0]

    from contextlib import ExitStack
    with ExitStack() as ctx:
        const = ctx.enter_context(tc.tile_pool(name="const", bufs=1))
        big = ctx.enter_context(tc.tile_pool(name="big", bufs=1))
        dram = ctx.enter_context(tc.tile_pool(name="dram", bufs=1, space="DRAM"))

        def load_const(name, shape, dtype):
            t = const.tile(list(shape), dtype, tag=name)
            nc.sync.dma_start(out=t[:], in_=ins[name][:])
            return t

        ident = load_const("ident", [P, P], BF16)
        iota_b = load_const("iota_b", [P, DST_BLK], BF16)
        e1w = load_const("e1w", [EF + 1, NF], BF16)
        e2w = load_const("e2w", [EF + 1, H], BF16)
        c1w1 = load_const("c1w1", [NF, H], BF16)
        c1w2 = load_const("c1w2", [H, H], BF16)
        c2w1 = load_const("c2w1", [H, H], BF16)
        c2w2 = load_const("c2w2", [H, H], BF16)
        aw1 = load_const("aw1", [H, H], BF16)
        aw2 = load_const("aw2", [H, A], BF16)
        nw1 = load_const("nw1", [H, H], BF16)
        nw2 = load_const("nw2", [H, H], BF16)
        nw3 = load_const("nw3", [H, 1], BF16)
        c1b1 = load_const("c1b1", [H, 1], F32)
        c1b2 = load_const("c1b2", [H, 1], F32)
        c2b1 = load_const("c2b1", [H, 1], F32)
        c2b2 = load_const("c2b2", [H, 1], F32)
        ab1 = load_const("ab1", [H, 1], F32)
        ab2 = load_const("ab2", [A, 1], F32)
        nb1 = load_const("nb1", [H, 1], F32)
        nb2 = load_const("nb2", [H, 1], F32)
        nb3 = load_const("nb3", [1, 1], F32)
        xT = load_const("xT", [NF, NC], F32)

        hpre = big.tile([NF, NC], BF16, tag="hpre")    # x + agg1 (channel-major)
        hT = big.tile([H, NC], BF16, tag="hT")         # relu(mlp1) local, channel-major
        h2pre = big.tile([H, NC], BF16, tag="h2pre")
        h2T = big.tile([H, NC], BF16, tag="h2T")
        nh = big.tile([H, NC], BF16, tag="nh")

        h_shard = dram.tile([NC, H], BF16, tag="h_shard")
        h_full = dram.tile([N, H], BF16, tag="h_full", addr_space="Shared")

        def gathers(gt3, it, table, ncols, Ce):
            """Issue <=1024-row dma_gather calls covering ncols chunk-columns."""
            off = 0
            while off < ncols:
                n = min(8, ncols - off)
                ni = n * P
                nc.gpsimd.dma_gather(
                    out_ap=gt3[:, off:off + n, :],
                    in_ap=table,
                    idxs_ap=it[:, off * 8:off * 8 + n * 8],
                    num_idxs=ni, num_idxs_reg=ni, elem_size=Ce,
                    queue_num=qctr[0] % NQ)
                qctr[0] += 1
                off += n

        def edge_phase(layer, block_done):
            C = NF if layer == 1 else H
            Ce = XCOL if layer == 1 else H          # gathered row width
            DT = F32 if layer == 1 else BF16
            ew = e1w if layer == 1 else e2w
            tbl = ins["x_pad"] if layer == 1 else h_full
            with ExitStack() as ectx:
                io = ectx.enter_context(tc.tile_pool(name=f"io{layer}", bufs=2))
                gtp = ectx.enter_context(tc.tile_pool(name=f"gtp{layer}", bufs=2))
                sb = ectx.enter_context(tc.tile_pool(name=f"sb{layer}", bufs=3))
                msgp = ectx.enter_context(
                    tc.tile_pool(name=f"msgp{layer}", bufs=3, space="PSUM"))
                aggp = ectx.enter_context(
                    tc.tile_pool(name=f"aggp{layer}", bufs=3, space="PSUM"))
                msb = ectx.enter_context(tc.tile_pool(name=f"m{layer}sb", bufs=3))
                mps = ectx.enter_context(
                    tc.tile_pool(name=f"m{layer}ps", bufs=2, space="PSUM"))
                for g in range(NG):
                    it_lo = io.tile([P, LO_COLS * 8], I16, tag="itlo")
                    nc.sync.dma_start(out=it_lo[:], in_=ins["idx_lo"][g])
                    it_hi = io.tile([P, HI_COLS * 8], I16, tag="ithi")
                    nc.sync.dma_start(out=it_hi[:], in_=ins["idx_hi"][g])
                    dl_t = io.tile([P, KG], BF16, tag="dl")
                    nc.sync.dma_start(out=dl_t[:], in_=ins["dstloc"][g])
                    ea_t = io.tile([EF + 1, KG * P], BF16, tag="ea")
                    nc.sync.dma_start(out=ea_t[:], in_=ins["eaT"][g])
                    gt = gtp.tile([P, KG * Ce], DT, tag="gt")
                    gt3 = gt[:].rearrange("p (n c) -> p n c", c=Ce)
                    gathers(gt3[:, 0:LO_COLS, :], it_lo, tbl[0:SPLIT, :], LO_COLS, Ce)
                    gathers(gt3[:, LO_COLS:KG, :], it_hi, tbl[SPLIT:N, :], HI_COLS, Ce)
                    for b in range(GB):
                        agg_t = aggp.tile([C, DST_BLK], F32, tag="agg")
                        # quads: runs of <=4 chunk-columns, class-pure & contiguous
                        quads = []
                        for c0 in range(0, CBL, 4):
                            quads.append((b * CBL + c0, min(4, CBL - c0)))
                        for c0 in range(0, CBH, 4):
                            quads.append((LO_COLS + b * CBH + c0, min(4, CBH - c0)))
                        ci = 0
                        for (col0, q) in quads:
                            mm = msgp.tile([P, 4 * C], F32, tag="mm")
                            msg_t = sb.tile([P, 4 * C], BF16, tag="msg")
                            for j in range(q):
                                nc.tensor.matmul(
                                    mm[:, j * C:(j + 1) * C],
                                    lhsT=ea_t[:, (col0 + j) * P:(col0 + j + 1) * P],
                                    rhs=ew[:], start=True, stop=True,
                                    skip_group_check=True)
                            tmp = sb.tile([P, 4 * C], F32, tag="tmp")
                            nc.vector.tensor_tensor(
                                out=tmp[:, 0:q * C],
                                in0=gt3[:, col0:col0 + q, 0:C],
                                in1=mm[:, 0:q * C], op=ALU.add)
                            nc.scalar.activation(msg_t[:, 0:q * C],
                                                 tmp[:, 0:q * C], AF.Relu)
                            for j in range(q):
                                col = col0 + j
                                oh_t = sb.tile([P, DST_BLK], BF16, tag="oh")
                                nc.vector.tensor_tensor(
                                    out=oh_t[:], in0=iota_b[:],
                                    in1=dl_t[:, col:col + 1].to_broadcast([P, DST_BLK]),
                                    op=ALU.is_equal)
                                nc.tensor.matmul(agg_t[:],
                                                 lhsT=msg_t[:, j * C:(j + 1) * C],
                                                 rhs=oh_t[:],
                                                 start=(ci == 0), stop=(ci == CB - 1))
                                ci += 1
                        colb = (g * GB + b) * DST_BLK
                        dst_sl = slice(colb, colb + DST_BLK)
                        if layer == 1:
                            nc.vector.tensor_tensor(
                                out=hpre[:, dst_sl], in0=xT[:, dst_sl],
                                in1=agg_t[:], op=ALU.add)
                        else:
                            nc.vector.tensor_tensor(
                                out=h2pre[:, dst_sl], in0=hT[:, dst_sl],
                                in1=agg_t[:], op=ALU.add)
                        block_done(g * GB + b, msb, mps)

        def leaky(pool, src_ps, bias, out_ap, width, parts=P):
            nc.scalar.activation(out_ap, src_ps[:], AF.Prelu,
                                 bias=bias[:, 0:1], alpha=0.01)

        # ---------------- layer 1 ----------------
        def mlp1_block(t, msb, mps):
            s = slice(t * P, (t + 1) * P)
            ps1 = mps.tile([H, P], F32, tag="ps")
            nc.tensor.matmul(ps1[:], lhsT=c1w1[:], rhs=hpre[:, s],
                             start=True, stop=True)
            l1 = msb.tile([H, P], BF16, tag="l1")
            leaky(msb, ps1, c1b1, l1[:], P)
            ps2 = mps.tile([H, P], F32, tag="ps")
            nc.tensor.matmul(ps2[:], lhsT=c1w2[:], rhs=l1[:],
                             start=True, stop=True)
            nc.scalar.activation(hT[:, s], ps2[:], AF.Relu, bias=c1b2[:, 0:1])
            tr = mps.tile([P, H], BF16, tag="ps")
            nc.tensor.transpose(tr[:], hT[:, s], ident[:])
            hrow = msb.tile([P, H], BF16, tag="hrow")
            nc.vector.tensor_copy(out=hrow[:], in_=tr[:])
            nc.sync.dma_start(out=h_shard[s, :], in_=hrow[:])

        edge_phase(1, mlp1_block)

        nc.gpsimd.collective_compute(
            "AllGather", ALU.bypass,
            replica_groups=[list(range(cfg["NCORES"]))],
            ins=[h_shard[:, :]], outs=[h_full[:, :]])

        # ---------------- layer 2 ----------------
        NS_STEP = min(512, NC)
        NS_BLKS = NS_STEP // P

        def mlp2_block(t, msb, mps):
            s = slice(t * P, (t + 1) * P)
            ps1 = mps.tile([H, P], F32, tag="ps")
            nc.tensor.matmul(ps1[:], lhsT=c2w1[:], rhs=h2pre[:, s],
                             start=True, stop=True)
            l1 = msb.tile([H, P], BF16, tag="l1")
            leaky(msb, ps1, c2b1, l1[:], P)
            ps2 = mps.tile([H, P], F32, tag="ps")
            nc.tensor.matmul(ps2[:], lhsT=c2w2[:], rhs=l1[:],
                             start=True, stop=True)
            leaky(msb, ps2, c2b2, h2T[:, s], P)
            ps3 = mps.tile([H, P], F32, tag="ps")
            nc.tensor.matmul(ps3[:], lhsT=nw1[:], rhs=h2T[:, s],
                             start=True, stop=True)
            n1 = msb.tile([H, P], BF16, tag="n1")
            leaky(msb, ps3, nb1, n1[:], P)
            ps4 = mps.tile([H, P], F32, tag="ps")
            nc.tensor.matmul(ps4[:], lhsT=nw2[:], rhs=n1[:],
                             start=True, stop=True)
            leaky(msb, ps4, nb2, nh[:, s], P)
            if (t + 1) % NS_BLKS == 0:
                s0 = (t + 1) * P - NS_STEP
                psn = mps.tile([1, NS_STEP], F32, tag="ps")
                nc.tensor.matmul(psn[:], lhsT=nw3[:], rhs=nh[:, s0:s0 + NS_STEP],
                                 start=True, stop=True)
                nst = msb.tile([1, NS_STEP], F32, tag="nst")
                nc.scalar.activation(nst[:], psn[:], AF.Sigmoid, bias=nb3[:, 0:1])
                nc.sync.dma_start(out=outs["out_ns"][0:1, s0:s0 + NS_STEP], in_=nst[:])

        edge_phase(2, mlp2_block)
        with ExitStack() as mctx:
            msb = mctx.enter_context(tc.tile_pool(name="ahsb", bufs=2))
            mps = mctx.enter_context(tc.tile_pool(name="ahps", bufs=2, space="PSUM"))
            # ---------------- action head ----------------
            pooled = msb.tile([H, GPC], F32, tag="pooled")
            for gi in range(GPC):
                nc.vector.tensor_reduce(
                    out=pooled[:, gi:gi + 1],
                    in_=h2T[:, gi * NPER:(gi + 1) * NPER],
                    axis=mybir.AxisListType.X, op=ALU.add)
            pooled_bf = msb.tile([H, GPC], BF16, tag="pooled_bf")
            nc.vector.tensor_scalar(out=pooled_bf[:], in0=pooled[:],
                                    scalar1=1.0 / NPER, scalar2=None, op0=ALU.mult)
            psa = mps.tile([H, GPC], F32, tag="ps")
            nc.tensor.matmul(psa[:], lhsT=aw1[:], rhs=pooled_bf[:],
                             start=True, stop=True)
            a1 = msb.tile([H, GPC], BF16, tag="a1")
            leaky(msb, psa, ab1, a1[:], GPC)
            psa2 = mps.tile([A, GPC], F32, tag="ps")
            nc.tensor.matmul(psa2[:], lhsT=aw2[:], rhs=a1[:],
                             start=True, stop=True)
            a2 = msb.tile([A, GPC], BF16, tag="a2")
            leaky(msb, psa2, ab2, a2[:], GPC, parts=A)
            trp = mps.tile([GPC, A], BF16, tag="ps")
            nc.tensor.transpose(trp[:], a2[:], ident[:A, :A])
            sm = msb.tile([GPC, A], F32, tag="sm")
            nc.vector.tensor_copy(out=sm[:], in_=trp[:])
            negmax = msb.tile([GPC, 1], F32, tag="negmax")
            nc.vector.tensor_reduce(out=negmax[:], in_=sm[:],
                                    axis=mybir.AxisListType.X, op=ALU.max,
                                    negate=True)
            ex = msb.tile([GPC, A], F32, tag="ex")
            nc.scalar.activation(ex[:], sm[:], AF.Exp, bias=negmax[:, 0:1])
            ssum = msb.tile([GPC, 1], F32, tag="ssum")
            nc.vector.tensor_reduce(out=ssum[:], in_=ex[:],
                                    axis=mybir.AxisListType.X, op=ALU.add)
            rinv = msb.tile([GPC, 1], F32, tag="rinv")
            nc.vector.reciprocal(rinv[:], ssum[:])
            act = msb.tile([GPC, A], F32, tag="act")
            nc.vector.tensor_scalar(out=act[:], in0=ex[:],
                                    scalar1=rinv[:, 0:1], scalar2=None,
                                    op0=ALU.mult)
            nc.sync.dma_start(out=outs["out_act"][:, :], in_=act[:])


# ---------------------------------------------------------------- driver
def build_program(ccfg):
    nc = bacc.Bacc("TRN2", target_bir_lowering=False, debug=False,
                   num_devices=ccfg["NCORES"], num_swdge_queues=NQ)
    N, NF, EF, H, A = ccfg["N"], ccfg["NF"], ccfg["EF"], ccfg["H"], ccfg["A"]
    NC, NG, KG, GPC = ccfg["NC"], ccfg["NG"], ccfg["KG"], ccfg["GPC"]
    DST_BLK, GB = ccfg["DST_BLK"], ccfg["GB"]
    LO_COLS, HI_COLS = GB * ccfg["CBL"], GB * ccfg["CBH"]

    def din(name, shape, dt):
        return nc.dram_tensor(name, list(shape), dt, kind="ExternalInput").ap()

    ins = dict(
        x_pad=din("x_pad", [N, XCOL], F32),
        xT=din("xT", [NF, NC], F32),
        idx_lo=din("idx_lo", [NG, P, LO_COLS * 8], I16),
        idx_hi=din("idx_hi", [NG, P, HI_COLS * 8], I16),
        dstloc=din("dstloc", [NG, P, KG], BF16),
        eaT=din("eaT", [NG, EF + 1, KG * P], BF16),
        ident=din("ident", [P, P], BF16),
        iota_b=din("iota_b", [P, DST_BLK], BF16),
        e1w=din("e1w", [EF + 1, NF], BF16),
        e2w=din("e2w", [EF + 1, H], BF16),
        c1w1=din("c1w1", [NF, H], BF16),
        c1w2=din("c1w2", [H, H], BF16),
        c2w1=din("c2w1", [H, H], BF16),
        c2w2=din("c2w2", [H, H], BF16),
        aw1=din("aw1", [H, H], BF16),
        aw2=din("aw2", [H, A], BF16),
        nw1=din("nw1", [H, H], BF16),
        nw2=din("nw2", [H, H], BF16),
        nw3=din("nw3", [H, 1], BF16),
        c1b1=din("c1b1", [H, 1], F32),
        c1b2=din("c1b2", [H, 1], F32),
        c2b1=din("c2b1", [H, 1], F32),
        c2b2=din("c2b2", [H, 1], F32),
        ab1=din("ab1", [H, 1], F32),
        ab2=din("ab2", [A, 1], F32),
        nb1=din("nb1", [H, 1], F32),
        nb2=din("nb2", [H, 1], F32),
        nb3=din("nb3", [1, 1], F32),
    )
    outs = dict(
        out_act=nc.dram_tensor("out_act", [GPC, A], F32, kind="ExternalOutput").ap(),
        out_ns=nc.dram_tensor("out_ns", [1, NC], F32, kind="ExternalOutput").ap(),
    )
    with tile.TileContext(nc) as tc:
        build_device_kernel(tc, outs, ins, ccfg)
    nc.compile()
    return nc


_PROGRAM_CACHE = {}


def assemble_outputs(cfg, results):
    B, A, NPER = cfg["B"], cfg["A"], cfg["NPER"]
    NCORES = cfg["NCORES"]
    action = np.concatenate([results[k]["out_act"] for k in range(NCORES)], 0)
    ns_all = np.concatenate(
        [results[k]["out_ns"].reshape(-1) for k in range(NCORES)], 0)
    node_scores = np.ascontiguousarray(ns_all.reshape(NPER, B).T)
    return action.astype(np.float32), node_scores.astype(np.float32)


def kernel(**inputs):
    cfg = default_cfg()
    ccfg, in_maps = prep_inputs(cfg, inputs)
    key = (ccfg["CBL"], ccfg["CBH"])
    if key not in _PROGRAM_CACHE:
        _PROGRAM_CACHE[key] = build_program(ccfg)
    nc = _PROGRAM_CACHE[key]
    res = bass_utils.run_bass_kernel_spmd(
        nc, in_maps, core_ids=list(range(ccfg["NCORES"])))
    return assemble_outputs(ccfg, res.results)
